# revision 33
# baseline (speedup 1.0000x reference)
"""BitLinear (BitNet-style) kernel for 8 Trainium2 NeuronCores.

Computes: out = input @ (sign(W) * mean(|W|)).T + bias
  input [8192, 2048] f32, W [8192, 2048] f32, bias [8192] f32 -> out [8192, 8192] f32

Sharding: column-parallel over out_features. Core j owns W rows
[j*1024, (j+1)*1024). Each core computes sign() on its shard (scalar
engine) and a local |W| partial sum (vector engine reduce with absolute
value); partial sums are AllReduce'd across the 8 cores so the scale is
the global abs-mean.

GEMM precision/speed: the PE's fp8 DoubleRow mode packs two k-planes per
matmul (stationary [128,2,M], moving [128,2,N]) and streams at 0.5
cycles per output column - 2x the bf16 column rate with twice the K per
step. sign(W) is exactly representable in fp8e4, and the input is fed as
an exact-ish hi+lo pair: x_hi = fp8(x), x_lo = fp8(x - x_hi), both
multiplied against the same sign stationary into the same PSUM
accumulation, recovering ~11 mantissa bits. The last N_SKIP_LO=3 of the
8 k-super-steps skip the lo correction (each skipped step adds
sqrt(1/8)*2.7e-2 in quadrature; measured 1.658e-2 end-to-end vs the
2e-2 gate) and save 1/16 of the PE time each.

Weights ship as fp8e4 of (W.T * 2048): sign is preserved (only |w| <
4.8e-7 quantizes to 0 - 119 of 16.7M elements, ~2e-3 quadrature error)
and the |W| partial sums come out scaled by 2048, folded into the final
scale constant. This halves weight DMA vs bf16 and gets the first
stationary ready sooner.

scale (fp32) and bias (fp32) are fused into the PSUM->SBUF eviction:
out = psum * scale + bias, written as bf16 (~1e-3 rounding, halves
store traffic); the host concatenates, transposes and upcasts.

Layout: host ships the input as two fp8 planes inH/inL of shape
[D_IN, TOKENS] (k-major). k is split (ks, i, p) = (super-step, DoubleRow
plane, partition): k = ks*256 + i*128 + p, a natural C-order reshape on
both operands so no host shuffling beyond the transpose.

Perf notes (cost-model profiled, 192.2 us vs 450.1 us bf16 baseline):
- 1664 DoubleRow matmuls of [K=256]x[M=128 o]x[N=512 t] at ~107 ns each
  (~178 us PE busy); the bf16 kernel's floor was ~438 us.
- 16 uniform 512-token spans: input arrives in 364 ns quanta so the PE
  is never waiting on a half-loaded 2048-token span; steady-state DMA
  per span (~7 us) is well under PE per span (~11 us). Total DMA
  ~128 us (input hi+lo 29 MB fp8, weights 2.1 MB fp8, output 16.7 MB
  bf16) vs the 360 B/ns ring.
- Prologue interleaves weight chunks with span-0 input loads on the SP
  ring; spans 0-1 run ks-outer (all 8 PSUM banks open) so the PE
  consumes each sign plane roughly as the ACT engine produces them
  (sign throughput, 1.9 us/plane, is the front-limiter: ~3 us of PE
  idle is paid waiting on the last planes; PE warmup matmuls cover the
  first ~4.7 us exactly).
- The first sign plane is produced in two o-halves so the first
  stationary is ready ~1 us sooner.
- Per-span staging tile [128, 8, 512] bf16 and batched SWDGE stores
  (two half-span DMAs per span): stores never sit on the ACT/SP
  sequencers where they would head-of-line block evictions (PSUM-bank
  back-pressure -> PE stall) or input loads. The last span stores per-o
  on the then-idle SP ring so the drain tail is one small DMA.
- The scale chain never touches the in-order PE queue: |W| partials on
  DVE, cross-partition fold via a DRAM bounce, broadcast via a step-0
  DMA, and its small DMAs ride the SWDGE queue.
- First 3 spans evict with a plain copy and fold scale+bias in a second
  DVE pass, so nothing stalls on the AllReduce latency.
"""

import sys

for _p in ("/opt/trn_rl_repo",):
    if _p not in sys.path:
        sys.path.append(_p)

import ml_dtypes
import numpy as np

TOKENS = 8192
D_IN = 2048
D_OUT = 8192
NCORES = 8
OSH = D_OUT // NCORES  # 1024 out features per core
P = 128
KS = D_IN // (2 * P)   # 8 k-super-tiles of 256 (two DoubleRow planes)
OT = OSH // P          # 8 o-tiles per core
SPAN = 512
NSPAN = TOKENS // SPAN
EARLY = 3              # spans evicted before the scale is known
N_SKIP_LO = 3          # k-super-steps (from the end) without lo correction
W_PRESCALE = 2048.0    # host premultiplier so fp8(W.T) keeps tiny signs

_NC_CACHE = {}


def _build_nc(use_collective=True, repeat=1, dedup_ldw=True,
              n_skip_lo=N_SKIP_LO):
    import concourse.mybir as mybir
    import concourse.tile as tile
    from concourse import bacc

    f32 = mybir.dt.float32
    bf16 = mybir.dt.bfloat16
    fp8 = mybir.dt.float8e4
    AF = mybir.ActivationFunctionType
    DR = mybir.MatmulPerfMode.DoubleRow

    nc = bacc.Bacc("TRN2", target_bir_lowering=False, debug=False,
                   num_devices=NCORES)

    inH = nc.dram_tensor("inH", [D_IN, TOKENS], fp8, kind="ExternalInput")
    inL = nc.dram_tensor("inL", [D_IN, TOKENS], fp8, kind="ExternalInput")
    wT = nc.dram_tensor("wT", [D_IN, OSH], fp8, kind="ExternalInput")
    bias2d = nc.dram_tensor("bias2d", [P, OT], f32, kind="ExternalInput")
    outT = nc.dram_tensor("outT", [OSH, TOKENS], bf16, kind="ExternalOutput")
    cc_in = nc.dram_tensor("cc_in", [1, 8], f32)
    cc_out = nc.dram_tensor("cc_out", [1, 8], f32, addr_space="Shared")
    colsum_dram = nc.dram_tensor("colsum_dram", [P], f32)

    # k = ks*256 + i*128 + p (natural C-order reshape)
    inH_r = inH.ap().rearrange("(ks i p) t -> p ks i t", i=2, p=P)
    inL_r = inL.ap().rearrange("(ks i p) t -> p ks i t", i=2, p=P)
    # kk = ks*2 + i: plane-major k-tile index of 128
    wT_r = wT.ap().rearrange("(kk p) o -> p kk o", p=P)
    outT_r = outT.ap().rearrange("(o p) t -> p o t", p=P)

    # W DMA schedule in k-super (256-k) units: small first loads so the
    # first stationary tiles are ready a couple of us in.
    if KS == 8:
        WSCHED = (1, 1, 2, 2, 2)
    else:
        WSCHED = (KS,)
    NWQ = len(WSCHED)
    WQMAX = max(WSCHED)

    with tile.TileContext(nc) as tc:
        with (
            tc.tile_pool(name="const", bufs=1) as const,
            tc.tile_pool(name="wpool", bufs=1) as wpool,
            tc.tile_pool(name="wstream", bufs=2) as wstream,
            tc.tile_pool(name="small", bufs=1) as small,
            tc.tile_pool(name="inpool", bufs=42) as inpool,
            tc.tile_pool(name="outpool", bufs=3) as outpool,
            tc.tile_pool(name="pmm", bufs=8, space="PSUM") as pmm,
        ):
            bias_sb = const.tile([P, OT], f32)
            nc.gpsimd.dma_start(bias_sb[:], bias2d.ap())

            # PE clock warmup: the HAM gate holds the array at 1.2 GHz until
            # ~3.4us of sustained activity. Burn that window on throwaway
            # matmuls over a zeroed tile while the first weights stream in,
            # so the real matmuls start at 2.4 GHz.
            warm_src = const.tile([P, 256], bf16)
            nc.vector.memset(warm_src[:], 0.0)
            warm_ps = pmm.tile([P, 512], f32, tag="mm", name="warm_ps")
            NWARM = 17
            for wmm in range(NWARM):
                nc.tensor.matmul(warm_ps[0:16, 0:256], warm_src[:, 0:16],
                                 warm_src[:],
                                 start=(wmm == 0), stop=(wmm == NWARM - 1))

            # --- weight shard: sign -> fp8 (DoubleRow layout), |W| partials ---
            # Prologue: weight chunks interleaved with span-0 input loads on
            # the SP ring, so sign planes and span-0 inputs arrive in the
            # order the ks-outer span-0 loop consumes them. All DMA issues
            # precede the signs; each sign only waits on its own chunk's
            # completion semaphore.
            # Sign on ACT; |.| row-sums on DVE; no PE involvement anywhere in
            # the scale chain so the in-order PE queue is never blocked on it.
            sT = wpool.tile([P, KS, 2, OSH], fp8)
            absacc = wpool.tile([P, NWQ], f32)

            def issue_in(q, t0, ks):
                ih = inpool.tile([P, 2, SPAN], fp8, tag="in",
                                 name=f"inh{q}_{ks}")
                nc.sync.dma_start(ih[:], inH_r[:, ks, :, t0:t0 + SPAN])
                il = None
                if ks < KS - n_skip_lo:
                    il = inpool.tile([P, 2, SPAN], fp8, tag="in",
                                     name=f"inl{q}_{ks}")
                    nc.sync.dma_start(il[:], inL_r[:, ks, :, t0:t0 + SPAN])
                return (ih, il)

            wts = []
            in0 = []
            k0 = 0
            for g, wq in enumerate(WSCHED):
                wt = wstream.tile([P, 2 * WQMAX, OSH], fp8, tag="wt",
                                  bufs=NWQ, name=f"wt{g}")
                nc.sync.dma_start(
                    wt[:, :2 * wq, :], wT_r[:, 2 * k0:2 * (k0 + wq), :]
                )
                wts.append((wt, k0, wq))
                for ks in range(k0, k0 + wq):
                    in0.append(issue_in(0, 0, ks))
                k0 += wq
            for g, (wt, k0, wq) in enumerate(wts):
                for s in range(wq):
                    ks = k0 + s
                    wsrc = wt[:, 2 * s:2 * s + 2, :]
                    if ks == 0:
                        # first plane in o-halves: the first stationary
                        # (ks0, o0) is ready ~1 us sooner
                        for h in range(2):
                            nc.scalar.activation(
                                sT[:, 0, :, h * 512:(h + 1) * 512],
                                wsrc[:, :, h * 512:(h + 1) * 512], AF.Sign)
                    else:
                        nc.scalar.activation(sT[:, ks, :, :], wsrc, AF.Sign)
                nc.vector.tensor_reduce(absacc[:, g:g + 1], wt[:, :2 * wq, :],
                                        axis=mybir.AxisListType.XY,
                                        op=mybir.AluOpType.add,
                                        apply_absolute_value=True)

            # --- global scale via AllReduce of the scalar partial ---
            # per-chunk |W| abs-row-sums live in absacc; fold the chunk axis
            # on DVE, then the partition axis via a DRAM bounce (the
            # partition axis can't fold into an SBUF free axis directly).
            # Scale is only needed by span EARLY's evictions (~50 us in).
            colsum = small.tile([P, 1], f32)
            nc.vector.reduce_sum(colsum[:], absacc[:],
                                 axis=mybir.AxisListType.X)
            nc.gpsimd.dma_start(colsum_dram.ap(), colsum[:, 0])
            rowt = small.tile([1, P], f32)
            nc.gpsimd.dma_start(rowt[0:1, :], colsum_dram.ap()[None, :])
            part = small.tile([1, 8], f32)
            nc.vector.memset(part[:], 0.0)
            nc.vector.reduce_sum(part[0:1, 0:1], rowt[0:1, :],
                                 axis=mybir.AxisListType.X)
            nc.gpsimd.dma_start(cc_in.ap(), part[:])
            if use_collective:
                nc.gpsimd.collective_compute(
                    "AllReduce",
                    mybir.AluOpType.add,
                    replica_groups=[list(range(NCORES))],
                    ins=[cc_in.ap()],
                    outs=[cc_out.ap()],
                )
                cc_result = cc_out
            else:
                # timing-model variant (TimelineSim can't model collectives):
                # local partial stands in for the global sum
                nc.gpsimd.dma_start(cc_out.ap(), cc_in.ap())
                cc_result = cc_out
            # broadcast the reduced scalar to all 128 partitions straight
            # from DRAM (step-0 source AP)
            scale_raw = small.tile([P, 1], f32)
            with nc.allow_non_contiguous_dma(reason="scale broadcast"):
                nc.gpsimd.dma_start(scale_raw[:, 0:1],
                                    cc_result.ap()[0:1, 0:1].to_broadcast((P, 1)))
            scale_b = small.tile([P, 1], f32)
            nc.scalar.activation(scale_b[:], scale_raw[:], AF.Copy,
                                 scale=1.0 / float(D_OUT * D_IN * W_PRESCALE))

            # --- main GEMM: outT[o, t] = sum_k sT[k, o] * (xhi+xlo)[k, t] ---
            # DoubleRow fp8: each matmul contracts 256 k (2 planes x 128
            # partitions) at 0.5 cycles per output column. hi and lo input
            # planes accumulate into the same PSUM bank; the sign stationary
            # is shared by both per (ks, o).
            spans = [(q + r * NSPAN, (q % NSPAN) * SPAN)
                     for r in range(repeat) for q in range(NSPAN)]
            nlo = KS - n_skip_lo

            def mm(ps, o, ks, pi, src, nparts):
                nc.tensor.matmul(
                    ps[:], sT[:, ks, :, o * P:(o + 1) * P], src[:],
                    start=(ks == 0 and pi == 0),
                    stop=(ks == KS - 1 and pi == nparts - 1),
                    perf_mode=DR,
                )

            def evict(stage, ps, o, early):
                if early:
                    # scale may still be in flight (the AllReduce) - plain
                    # copy frees the PSUM bank now; scale+bias folded in a
                    # DVE pass afterwards.
                    nc.scalar.activation(stage[:, o, :], ps[:], AF.Copy)
                    nc.vector.tensor_scalar(
                        stage[:, o, :], stage[:, o, :],
                        scale_b[:, 0:1], bias_sb[:, o:o + 1],
                        mybir.AluOpType.mult, mybir.AluOpType.add)
                else:
                    nc.scalar.activation(
                        stage[:, o, :], ps[:], AF.Identity,
                        bias=bias_sb[:, o:o + 1], scale=scale_b[:, 0:1])

            for q, t0 in spans:
                early = q < EARLY
                if q == 0:
                    inq = in0
                else:
                    inq = [issue_in(q, t0, ks) for ks in range(KS)]
                stage = outpool.tile([P, OT, SPAN], bf16, tag="stage",
                                     name=f"st{q}")
                psums = [pmm.tile([P, SPAN], f32, tag="mm",
                                  name=f"pp{q}_{o}") for o in range(OT)]
                if q < 2:
                    # ks-outer: consume each sign plane / input tile as it's
                    # produced; all 8 PSUM banks accumulate simultaneously.
                    # At the last ks, finish + evict per o so banks free for
                    # the next span as the ACT engine catches up.
                    for ks in range(KS - 1):
                        for o in range(OT):
                            for pi, src in enumerate(
                                    s for s in inq[ks] if s is not None):
                                mm(psums[o], o, ks, pi, src,
                                   2 if ks < nlo else 1)
                    ks = KS - 1
                    for o in range(OT):
                        for pi, src in enumerate(
                                s for s in inq[ks] if s is not None):
                            mm(psums[o], o, ks, pi, src, 2 if ks < nlo else 1)
                        evict(stage, psums[o], o, early)
                else:
                    last = q == spans[-1][0]
                    for o in range(OT):
                        for ks in range(KS):
                            for pi, src in enumerate(
                                    s for s in inq[ks] if s is not None):
                                mm(psums[o], o, ks, pi, src,
                                   2 if ks < nlo else 1)
                        evict(stage, psums[o], o, early)
                        if last:
                            # per-o stores right behind each eviction, on
                            # the SP ring - input loads are done by now, so
                            # SP is idle and the ACT queue keeps evicting:
                            # the drain tail is one small DMA
                            nc.sync.dma_start(outT_r[:, o, t0:t0 + SPAN],
                                              stage[:, o, :])
                    if last:
                        continue
                # batched stores per span half on the SWDGE queue (two
                # ~1.5 us device slices interleave with input loads better
                # than one 3 us one)
                h = OT // 2
                nc.gpsimd.dma_start(outT_r[:, 0:h, t0:t0 + SPAN],
                                    stage[:, 0:h, :])
                nc.gpsimd.dma_start(outT_r[:, h:, t0:t0 + SPAN],
                                    stage[:, h:, :])

    if dedup_ldw:
        _dedup_ldweights(nc, mybir)
    nc.compile()
    return nc


def _dedup_ldweights(nc, mybir):
    """Drop consecutive InstLdweights that reload the exact same stationary
    AP with only matmuls in between. Tile emits one weight load per matmul
    even when several matmuls share a stationary; the following
    non-self-loading matmuls keep using the already-loaded array state.
    Only waitless/updateless loads are removed."""
    removed = 0
    for bb in nc.m.functions[0].blocks:
        il = bb.instructions
        kept = []
        prev_sig = None
        for i in il:
            if isinstance(i, mybir.InstLdweights):
                sig = str(i.ins[0])
                if (sig == prev_sig and not i.has_wait()
                        and not i.has_update()):
                    nc.inst_map.pop(i.name, None)
                    removed += 1
                    continue
                prev_sig = sig
            elif isinstance(i, mybir.InstMatmult):
                pass
            elif getattr(i, "engine", None) == mybir.EngineType.PE:
                prev_sig = None
            kept.append(i)
        il[:] = kept


def _get_nc():
    if "nc" not in _NC_CACHE:
        _NC_CACHE["nc"] = _build_nc()
    return _NC_CACHE["nc"]


def _make_in_maps(input, weight, bias):
    inT = np.ascontiguousarray(input.T)
    inT_hi = inT.astype(ml_dtypes.float8_e4m3)
    inT_lo = (inT - inT_hi.astype(np.float32)).astype(ml_dtypes.float8_e4m3)
    wT_full = weight.T  # [D_IN, D_OUT] view
    in_maps = []
    for j in range(NCORES):
        bsh = bias[j * OSH:(j + 1) * OSH]
        in_maps.append({
            "inH": inT_hi,
            "inL": inT_lo,
            "wT": (np.ascontiguousarray(wT_full[:, j * OSH:(j + 1) * OSH])
                   * np.float32(W_PRESCALE)).astype(ml_dtypes.float8_e4m3),
            "bias2d": np.ascontiguousarray(
                bsh.reshape(OT, P).T, dtype=np.float32),
        })
    return in_maps


def run(input, weight, bias, trace=False, **spmd_kwargs):
    from concourse.bass_utils import run_bass_kernel_spmd

    nc = _get_nc()
    in_maps = _make_in_maps(np.asarray(input, dtype=np.float32),
                            np.asarray(weight, dtype=np.float32),
                            np.asarray(bias, dtype=np.float32))
    res = run_bass_kernel_spmd(nc, in_maps, core_ids=list(range(NCORES)),
                               trace=trace, **spmd_kwargs)
    outT = np.concatenate([r["outT"] for r in res.results], axis=0)
    out = np.ascontiguousarray(outT.T.astype(np.float32))
    return out, res


def kernel(input, weight, bias):
    out, _ = run(input, weight, bias, trace=False)
    return out


# revision 35
# speedup vs baseline: 1.0108x; 1.0108x over previous
"""BitLinear (BitNet-style) kernel for 8 Trainium2 NeuronCores.

Computes: out = input @ (sign(W) * mean(|W|)).T + bias
  input [8192, 2048] f32, W [8192, 2048] f32, bias [8192] f32 -> out [8192, 8192] f32

Sharding: column-parallel over out_features. Core j owns W rows
[j*1024, (j+1)*1024). Each core computes sign() on its shard (scalar
engine) and a local |W| partial sum (vector engine reduce with absolute
value); partial sums are AllReduce'd across the 8 cores so the scale is
the global abs-mean.

GEMM precision/speed: the PE's fp8 DoubleRow mode packs two k-planes per
matmul (stationary [128,2,M], moving [128,2,N]) and streams at 0.5
cycles per output column - 2x the bf16 column rate with twice the K per
step. sign(W) is exactly representable in fp8e4, and the input is fed as
an exact-ish hi+lo pair: x_hi = fp8(x), x_lo = fp8(x - x_hi), both
multiplied against the same sign stationary into the same PSUM
accumulation, recovering ~11 mantissa bits. The last N_SKIP_LO=3 of the
8 k-super-steps skip the lo correction (each skipped step adds
sqrt(1/8)*2.7e-2 in quadrature; measured 1.658e-2 end-to-end vs the
2e-2 gate) and save 1/16 of the PE time each.

Weights ship as fp8e4 of (W.T * 2048): sign is preserved (only |w| <
4.8e-7 quantizes to 0 - 119 of 16.7M elements, ~2e-3 quadrature error)
and the |W| partial sums come out scaled by 2048, folded into the final
scale constant. This halves weight DMA vs bf16 and gets the first
stationary ready sooner.

scale (fp32) and bias (fp32) are fused into the PSUM->SBUF eviction:
out = psum * scale + bias, written as bf16 (~1e-3 rounding, halves
store traffic); the host concatenates, transposes and upcasts.

Layout: host ships the input as two fp8 planes inH/inL of shape
[D_IN, TOKENS] (k-major). k is split (ks, i, p) = (super-step, DoubleRow
plane, partition): k = ks*256 + i*128 + p, a natural C-order reshape on
both operands so no host shuffling beyond the transpose.

Perf notes (cost-model profiled, 192.2 us vs 450.1 us bf16 baseline):
- 1664 DoubleRow matmuls of [K=256]x[M=128 o]x[N=512 t] at ~107 ns each
  (~178 us PE busy); the bf16 kernel's floor was ~438 us.
- 16 uniform 512-token spans: input arrives in 364 ns quanta so the PE
  is never waiting on a half-loaded 2048-token span; steady-state DMA
  per span (~7 us) is well under PE per span (~11 us). Total DMA
  ~128 us (input hi+lo 29 MB fp8, weights 2.1 MB fp8, output 16.7 MB
  bf16) vs the 360 B/ns ring.
- Prologue interleaves weight chunks with span-0 input loads on the SP
  ring; spans 0-1 run ks-outer (all 8 PSUM banks open) so the PE
  consumes each sign plane roughly as the ACT engine produces them
  (sign throughput, 1.9 us/plane, is the front-limiter: ~3 us of PE
  idle is paid waiting on the last planes; PE warmup matmuls cover the
  first ~4.7 us exactly).
- The first sign plane is produced in two o-halves so the first
  stationary is ready ~1 us sooner.
- Per-span staging tile [128, 8, 512] bf16 and batched SWDGE stores
  (two half-span DMAs per span): stores never sit on the ACT/SP
  sequencers where they would head-of-line block evictions (PSUM-bank
  back-pressure -> PE stall) or input loads. The last span stores per-o
  on the then-idle SP ring so the drain tail is one small DMA.
- The scale chain never touches the in-order PE queue: |W| partials on
  DVE, cross-partition fold via a DRAM bounce, broadcast via a step-0
  DMA, and its small DMAs ride the SWDGE queue.
- First 3 spans evict with a plain copy and fold scale+bias in a second
  DVE pass, so nothing stalls on the AllReduce latency.
"""

import sys

for _p in ("/opt/trn_rl_repo",):
    if _p not in sys.path:
        sys.path.append(_p)

import ml_dtypes
import numpy as np

TOKENS = 8192
D_IN = 2048
D_OUT = 8192
NCORES = 8
OSH = D_OUT // NCORES  # 1024 out features per core
P = 128
KS = D_IN // (2 * P)   # 8 k-super-tiles of 256 (two DoubleRow planes)
OT = OSH // P          # 8 o-tiles per core
SPAN = 512
NSPAN = TOKENS // SPAN
EARLY = 3              # spans evicted before the scale is known
N_SKIP_LO = 3          # k-super-steps (from the end) without lo correction
W_PRESCALE = 2048.0    # host premultiplier so fp8(W.T) keeps tiny signs

_NC_CACHE = {}


def _build_nc(use_collective=True, repeat=1, dedup_ldw=True,
              n_skip_lo=N_SKIP_LO):
    import concourse.mybir as mybir
    import concourse.tile as tile
    from concourse import bacc

    f32 = mybir.dt.float32
    bf16 = mybir.dt.bfloat16
    fp8 = mybir.dt.float8e4
    AF = mybir.ActivationFunctionType
    DR = mybir.MatmulPerfMode.DoubleRow

    nc = bacc.Bacc("TRN2", target_bir_lowering=False, debug=False,
                   num_devices=NCORES)

    inH = nc.dram_tensor("inH", [D_IN, TOKENS], fp8, kind="ExternalInput")
    inL = nc.dram_tensor("inL", [D_IN, TOKENS], fp8, kind="ExternalInput")
    wT = nc.dram_tensor("wT", [D_IN, OSH], fp8, kind="ExternalInput")
    bias2d = nc.dram_tensor("bias2d", [P, OT], f32, kind="ExternalInput")
    outT = nc.dram_tensor("outT", [OSH, TOKENS], bf16, kind="ExternalOutput")
    cc_in = nc.dram_tensor("cc_in", [1, 8], f32)
    cc_out = nc.dram_tensor("cc_out", [1, 8], f32, addr_space="Shared")
    colsum_dram = nc.dram_tensor("colsum_dram", [P], f32)

    # k = ks*256 + i*128 + p (natural C-order reshape)
    inH_r = inH.ap().rearrange("(ks i p) t -> p ks i t", i=2, p=P)
    inL_r = inL.ap().rearrange("(ks i p) t -> p ks i t", i=2, p=P)
    # kk = ks*2 + i: plane-major k-tile index of 128
    wT_r = wT.ap().rearrange("(kk p) o -> p kk o", p=P)
    outT_r = outT.ap().rearrange("(o p) t -> p o t", p=P)

    # W DMA schedule in k-super (256-k) units: small first loads so the
    # first stationary tiles are ready a couple of us in.
    if KS == 8:
        WSCHED = (1, 1, 2, 2, 2)
    else:
        WSCHED = (KS,)
    NWQ = len(WSCHED)
    WQMAX = max(WSCHED)

    with tile.TileContext(nc) as tc:
        with (
            tc.tile_pool(name="const", bufs=1) as const,
            tc.tile_pool(name="wpool", bufs=1) as wpool,
            tc.tile_pool(name="wstream", bufs=2) as wstream,
            tc.tile_pool(name="small", bufs=1) as small,
            tc.tile_pool(name="inpool", bufs=42) as inpool,
            tc.tile_pool(name="outpool", bufs=5) as outpool,
            tc.tile_pool(name="pmm", bufs=8, space="PSUM") as pmm,
        ):
            bias_sb = const.tile([P, OT], f32)
            nc.gpsimd.dma_start(bias_sb[:], bias2d.ap())

            # PE clock warmup: the HAM gate holds the array at 1.2 GHz until
            # ~3.4us of sustained activity. Burn that window on throwaway
            # matmuls over a zeroed tile while the first weights stream in,
            # so the real matmuls start at 2.4 GHz.
            warm_src = const.tile([P, 256], bf16)
            nc.vector.memset(warm_src[:], 0.0)
            warm_ps = pmm.tile([P, 512], f32, tag="mm", name="warm_ps")
            NWARM = 17
            for wmm in range(NWARM):
                nc.tensor.matmul(warm_ps[0:16, 0:256], warm_src[:, 0:16],
                                 warm_src[:],
                                 start=(wmm == 0), stop=(wmm == NWARM - 1))

            # --- weight shard: sign -> fp8 (DoubleRow layout), |W| partials ---
            # Prologue: weight chunks interleaved with span-0 input loads on
            # the SP ring, so sign planes and span-0 inputs arrive in the
            # order the ks-outer span-0 loop consumes them. All DMA issues
            # precede the signs; each sign only waits on its own chunk's
            # completion semaphore.
            # Sign on ACT; |.| row-sums on DVE; no PE involvement anywhere in
            # the scale chain so the in-order PE queue is never blocked on it.
            sT = wpool.tile([P, KS, 2, OSH], fp8)
            absacc = wpool.tile([P, NWQ], f32)

            def issue_in(q, t0, ks):
                ih = inpool.tile([P, 2, SPAN], fp8, tag="in",
                                 name=f"inh{q}_{ks}")
                nc.sync.dma_start(ih[:], inH_r[:, ks, :, t0:t0 + SPAN])
                il = None
                if ks < KS - n_skip_lo:
                    il = inpool.tile([P, 2, SPAN], fp8, tag="in",
                                     name=f"inl{q}_{ks}")
                    nc.sync.dma_start(il[:], inL_r[:, ks, :, t0:t0 + SPAN])
                return (ih, il)

            wts = []
            in0 = []
            k0 = 0
            deferred_in = []
            for g, wq in enumerate(WSCHED):
                wt = wstream.tile([P, 2 * WQMAX, OSH], fp8, tag="wt",
                                  bufs=NWQ, name=f"wt{g}")
                nc.sync.dma_start(
                    wt[:, :2 * wq, :], wT_r[:, 2 * k0:2 * (k0 + wq), :]
                )
                wts.append((wt, k0, wq))
                if g >= NWQ - 2:
                    # last two weight chunks ride back-to-back so the last
                    # sign planes can start before the tail input loads
                    deferred_in.extend(range(k0, k0 + wq))
                else:
                    for ks in range(k0, k0 + wq):
                        in0.append(issue_in(0, 0, ks))
                k0 += wq
            for ks in deferred_in:
                in0.append(issue_in(0, 0, ks))
            # sign planes alternate ACT (even ks) / DVE (odd ks, as
            # multiply-clamp: the shipped fp8 weights' smallest nonzero
            # magnitude is 2^-9 * 2048-prescale units, so x*1e30 saturates
            # far past +-1 and zeros stay zero). Two engines stream planes
            # ~2x faster than one, tracking the ks-outer span-0 consumption.
            for g, (wt, k0, wq) in enumerate(wts):
                for s in range(wq):
                    ks = k0 + s
                    wsrc = wt[:, 2 * s:2 * s + 2, :]
                    if ks == 0:
                        # first plane in o-halves: the first stationary
                        # (ks0, o0) is ready ~1 us sooner
                        for h in range(2):
                            nc.scalar.activation(
                                sT[:, 0, :, h * 512:(h + 1) * 512],
                                wsrc[:, :, h * 512:(h + 1) * 512], AF.Sign)
                    elif ks % 2 == 0:
                        nc.scalar.activation(sT[:, ks, :, :], wsrc, AF.Sign)
                    else:
                        stmp = wstream.tile([P, 2, OSH], bf16, tag="stmp",
                                            bufs=2, name=f"stmp{ks}")
                        nc.vector.tensor_scalar(
                            stmp[:], wsrc, 1e30, 1.0,
                            mybir.AluOpType.mult, mybir.AluOpType.min)
                        nc.vector.tensor_scalar(
                            sT[:, ks, :, :], stmp[:], -1.0, None,
                            mybir.AluOpType.max)
            # |W| partials pinned past the sign stream (tile_wait_until):
            # they are ready before the DVE sign ops and would otherwise be
            # scheduled ahead of them; the scale isn't needed until span
            # EARLY's evictions (~50 us), so a 16 us floor costs nothing.
            with tc.tile_wait_until(0.016):
                for g, (wt, k0, wq) in enumerate(wts):
                    nc.vector.tensor_reduce(absacc[:, g:g + 1],
                                            wt[:, :2 * wq, :],
                                            axis=mybir.AxisListType.XY,
                                            op=mybir.AluOpType.add,
                                            apply_absolute_value=True)

            # --- global scale via AllReduce of the scalar partial ---
            # per-chunk |W| abs-row-sums live in absacc; fold the chunk axis
            # on DVE, then the partition axis via a DRAM bounce (the
            # partition axis can't fold into an SBUF free axis directly).
            # Scale is only needed by span EARLY's evictions (~50 us in).
            colsum = small.tile([P, 1], f32)
            nc.vector.reduce_sum(colsum[:], absacc[:],
                                 axis=mybir.AxisListType.X)
            nc.gpsimd.dma_start(colsum_dram.ap(), colsum[:, 0])
            rowt = small.tile([1, P], f32)
            nc.gpsimd.dma_start(rowt[0:1, :], colsum_dram.ap()[None, :])
            part = small.tile([1, 8], f32)
            nc.vector.memset(part[:], 0.0)
            nc.vector.reduce_sum(part[0:1, 0:1], rowt[0:1, :],
                                 axis=mybir.AxisListType.X)
            nc.gpsimd.dma_start(cc_in.ap(), part[:])
            if use_collective:
                nc.gpsimd.collective_compute(
                    "AllReduce",
                    mybir.AluOpType.add,
                    replica_groups=[list(range(NCORES))],
                    ins=[cc_in.ap()],
                    outs=[cc_out.ap()],
                )
                cc_result = cc_out
            else:
                # timing-model variant (TimelineSim can't model collectives):
                # local partial stands in for the global sum
                nc.gpsimd.dma_start(cc_out.ap(), cc_in.ap())
                cc_result = cc_out
            # broadcast the reduced scalar to all 128 partitions straight
            # from DRAM (step-0 source AP)
            scale_raw = small.tile([P, 1], f32)
            with nc.allow_non_contiguous_dma(reason="scale broadcast"):
                nc.gpsimd.dma_start(scale_raw[:, 0:1],
                                    cc_result.ap()[0:1, 0:1].to_broadcast((P, 1)))
            scale_b = small.tile([P, 1], f32)
            nc.scalar.activation(scale_b[:], scale_raw[:], AF.Copy,
                                 scale=1.0 / float(D_OUT * D_IN * W_PRESCALE))

            # --- main GEMM: outT[o, t] = sum_k sT[k, o] * (xhi+xlo)[k, t] ---
            # DoubleRow fp8: each matmul contracts 256 k (2 planes x 128
            # partitions) at 0.5 cycles per output column. hi and lo input
            # planes accumulate into the same PSUM bank; the sign stationary
            # is shared by both per (ks, o).
            spans = [(q + r * NSPAN, (q % NSPAN) * SPAN)
                     for r in range(repeat) for q in range(NSPAN)]
            nlo = KS - n_skip_lo

            def mm(ps, o, ks, pi, src, nparts):
                nc.tensor.matmul(
                    ps[:], sT[:, ks, :, o * P:(o + 1) * P], src[:],
                    start=(ks == 0 and pi == 0),
                    stop=(ks == KS - 1 and pi == nparts - 1),
                    perf_mode=DR,
                )

            def evict(stage, ps, o, early):
                if early:
                    # scale may still be in flight (the AllReduce) - plain
                    # copy frees the PSUM bank now; scale+bias folded in a
                    # DVE pass afterwards.
                    nc.scalar.activation(stage[:, o, :], ps[:], AF.Copy)
                    nc.vector.tensor_scalar(
                        stage[:, o, :], stage[:, o, :],
                        scale_b[:, 0:1], bias_sb[:, o:o + 1],
                        mybir.AluOpType.mult, mybir.AluOpType.add)
                else:
                    nc.scalar.activation(
                        stage[:, o, :], ps[:], AF.Identity,
                        bias=bias_sb[:, o:o + 1], scale=scale_b[:, 0:1])

            for q, t0 in spans:
                early = q < EARLY
                if q == 0:
                    inq = in0
                else:
                    inq = [issue_in(q, t0, ks) for ks in range(KS)]
                stage = outpool.tile([P, OT, SPAN], bf16, tag="stage",
                                     name=f"st{q}")
                psums = [pmm.tile([P, SPAN], f32, tag="mm",
                                  name=f"pp{q}_{o}") for o in range(OT)]
                if q < 2:
                    # ks-outer: consume each sign plane / input tile as it's
                    # produced; all 8 PSUM banks accumulate simultaneously.
                    # At the last ks, finish + evict per o so banks free for
                    # the next span as the ACT engine catches up.
                    for ks in range(KS - 1):
                        for o in range(OT):
                            for pi, src in enumerate(
                                    s for s in inq[ks] if s is not None):
                                mm(psums[o], o, ks, pi, src,
                                   2 if ks < nlo else 1)
                    ks = KS - 1
                    for o in range(OT):
                        for pi, src in enumerate(
                                s for s in inq[ks] if s is not None):
                            mm(psums[o], o, ks, pi, src, 2 if ks < nlo else 1)
                        evict(stage, psums[o], o, early)
                else:
                    last = q == spans[-1][0]
                    for o in range(OT):
                        for ks in range(KS):
                            for pi, src in enumerate(
                                    s for s in inq[ks] if s is not None):
                                mm(psums[o], o, ks, pi, src,
                                   2 if ks < nlo else 1)
                        evict(stage, psums[o], o, early)
                        if last:
                            # per-o stores right behind each eviction, on
                            # the SP ring - input loads are done by now, so
                            # SP is idle and the ACT queue keeps evicting:
                            # the drain tail is one small DMA
                            nc.sync.dma_start(outT_r[:, o, t0:t0 + SPAN],
                                              stage[:, o, :])
                    if last:
                        continue
                # batched stores per span half on the SWDGE queue (two
                # ~1.5 us device slices interleave with input loads better
                # than one 3 us one)
                h = OT // 2
                nc.gpsimd.dma_start(outT_r[:, 0:h, t0:t0 + SPAN],
                                    stage[:, 0:h, :])
                nc.gpsimd.dma_start(outT_r[:, h:, t0:t0 + SPAN],
                                    stage[:, h:, :])

    if dedup_ldw:
        _dedup_ldweights(nc, mybir)
    nc.compile()
    return nc


def _dedup_ldweights(nc, mybir):
    """Drop consecutive InstLdweights that reload the exact same stationary
    AP with only matmuls in between. Tile emits one weight load per matmul
    even when several matmuls share a stationary; the following
    non-self-loading matmuls keep using the already-loaded array state.
    Only waitless/updateless loads are removed."""
    removed = 0
    for bb in nc.m.functions[0].blocks:
        il = bb.instructions
        kept = []
        prev_sig = None
        for i in il:
            if isinstance(i, mybir.InstLdweights):
                sig = str(i.ins[0])
                if (sig == prev_sig and not i.has_wait()
                        and not i.has_update()):
                    nc.inst_map.pop(i.name, None)
                    removed += 1
                    continue
                prev_sig = sig
            elif isinstance(i, mybir.InstMatmult):
                pass
            elif getattr(i, "engine", None) == mybir.EngineType.PE:
                prev_sig = None
            kept.append(i)
        il[:] = kept


def _get_nc():
    if "nc" not in _NC_CACHE:
        _NC_CACHE["nc"] = _build_nc()
    return _NC_CACHE["nc"]


def _make_in_maps(input, weight, bias):
    inT = np.ascontiguousarray(input.T)
    inT_hi = inT.astype(ml_dtypes.float8_e4m3)
    inT_lo = (inT - inT_hi.astype(np.float32)).astype(ml_dtypes.float8_e4m3)
    wT_full = weight.T  # [D_IN, D_OUT] view
    in_maps = []
    for j in range(NCORES):
        bsh = bias[j * OSH:(j + 1) * OSH]
        in_maps.append({
            "inH": inT_hi,
            "inL": inT_lo,
            "wT": (np.ascontiguousarray(wT_full[:, j * OSH:(j + 1) * OSH])
                   * np.float32(W_PRESCALE)).astype(ml_dtypes.float8_e4m3),
            "bias2d": np.ascontiguousarray(
                bsh.reshape(OT, P).T, dtype=np.float32),
        })
    return in_maps


def run(input, weight, bias, trace=False, **spmd_kwargs):
    from concourse.bass_utils import run_bass_kernel_spmd

    nc = _get_nc()
    in_maps = _make_in_maps(np.asarray(input, dtype=np.float32),
                            np.asarray(weight, dtype=np.float32),
                            np.asarray(bias, dtype=np.float32))
    res = run_bass_kernel_spmd(nc, in_maps, core_ids=list(range(NCORES)),
                               trace=trace, **spmd_kwargs)
    outT = np.concatenate([r["outT"] for r in res.results], axis=0)
    out = np.ascontiguousarray(outT.T.astype(np.float32))
    return out, res


def kernel(input, weight, bias):
    out, _ = run(input, weight, bias, trace=False)
    return out


# revision 37
# speedup vs baseline: 1.0817x; 1.0701x over previous
"""BitLinear (BitNet-style) kernel for 8 Trainium2 NeuronCores.

Computes: out = input @ (sign(W) * mean(|W|)).T + bias
  input [8192, 2048] f32, W [8192, 2048] f32, bias [8192] f32 -> out [8192, 8192] f32

Sharding: column-parallel over out_features. Core j owns W rows
[j*1024, (j+1)*1024). Each core computes sign() on its shard (scalar
engine) and a local |W| partial sum (vector engine reduce with absolute
value); partial sums are AllReduce'd across the 8 cores so the scale is
the global abs-mean.

GEMM precision/speed: the PE's fp8 DoubleRow mode packs two k-planes per
matmul (stationary [128,2,M], moving [128,2,N]) and streams at 0.5
cycles per output column - 2x the bf16 column rate with twice the K per
step. sign(W) is exactly representable in fp8e4, and the input is fed as
an exact-ish hi+lo pair: x_hi = fp8(x), x_lo = fp8(x - x_hi), both
multiplied against the same sign stationary into the same PSUM
accumulation, recovering ~11 mantissa bits. The last N_SKIP_LO=3 of the
8 k-super-steps skip the lo correction (each skipped step adds
sqrt(1/8)*2.7e-2 in quadrature; measured 1.658e-2 end-to-end vs the
2e-2 gate) and save 1/16 of the PE time each.

Weights ship as fp8e4 of (W.T * 2048): sign is preserved (only |w| <
4.8e-7 quantizes to 0 - 119 of 16.7M elements, ~2e-3 quadrature error)
and the |W| partial sums come out scaled by 2048, folded into the final
scale constant. This halves weight DMA vs bf16 and gets the first
stationary ready sooner.

scale (fp32) and bias (fp32) are fused into the PSUM->SBUF eviction:
out = psum * scale + bias, written as bf16 (~1e-3 rounding, halves
store traffic); the host concatenates, transposes and upcasts.

Layout: host ships the input as two fp8 planes inH/inL of shape
[D_IN, TOKENS] (k-major). k is split (ks, i, p) = (super-step, DoubleRow
plane, partition): k = ks*256 + i*128 + p, a natural C-order reshape on
both operands so no host shuffling beyond the transpose.

Perf notes (cost-model profiled, 192.2 us vs 450.1 us bf16 baseline):
- 1664 DoubleRow matmuls of [K=256]x[M=128 o]x[N=512 t] at ~107 ns each
  (~178 us PE busy); the bf16 kernel's floor was ~438 us.
- 16 uniform 512-token spans: input arrives in 364 ns quanta so the PE
  is never waiting on a half-loaded 2048-token span; steady-state DMA
  per span (~7 us) is well under PE per span (~11 us). Total DMA
  ~128 us (input hi+lo 29 MB fp8, weights 2.1 MB fp8, output 16.7 MB
  bf16) vs the 360 B/ns ring.
- Prologue interleaves weight chunks with span-0 input loads on the SP
  ring; spans 0-1 run ks-outer (all 8 PSUM banks open) so the PE
  consumes each sign plane roughly as the ACT engine produces them
  (sign throughput, 1.9 us/plane, is the front-limiter: ~3 us of PE
  idle is paid waiting on the last planes; PE warmup matmuls cover the
  first ~4.7 us exactly).
- The first sign plane is produced in two o-halves so the first
  stationary is ready ~1 us sooner.
- Per-span staging tile [128, 8, 512] bf16 and batched SWDGE stores
  (two half-span DMAs per span): stores never sit on the ACT/SP
  sequencers where they would head-of-line block evictions (PSUM-bank
  back-pressure -> PE stall) or input loads. The last span stores per-o
  on the then-idle SP ring so the drain tail is one small DMA.
- The scale chain never touches the in-order PE queue: |W| partials on
  DVE, cross-partition fold via a DRAM bounce, broadcast via a step-0
  DMA, and its small DMAs ride the SWDGE queue.
- First 3 spans evict with a plain copy and fold scale+bias in a second
  DVE pass, so nothing stalls on the AllReduce latency.
"""

import sys

for _p in ("/opt/trn_rl_repo",):
    if _p not in sys.path:
        sys.path.append(_p)

import ml_dtypes
import numpy as np

TOKENS = 8192
D_IN = 2048
D_OUT = 8192
NCORES = 8
OSH = D_OUT // NCORES  # 1024 out features per core
P = 128
KS = D_IN // (2 * P)   # 8 k-super-tiles of 256 (two DoubleRow planes)
OT = OSH // P          # 8 o-tiles per core
SPAN = 512
NSPAN = TOKENS // SPAN
EARLY = 3              # spans evicted before the scale is known
N_SKIP_LO = 4          # k-super-steps (from the end) without lo correction
W_PRESCALE = 2048.0    # host premultiplier so fp8(W.T) keeps tiny signs

_NC_CACHE = {}


def _build_nc(use_collective=True, repeat=1, dedup_ldw=True,
              n_skip_lo=N_SKIP_LO):
    import concourse.mybir as mybir
    import concourse.tile as tile
    from concourse import bacc

    f32 = mybir.dt.float32
    bf16 = mybir.dt.bfloat16
    fp8 = mybir.dt.float8e4
    AF = mybir.ActivationFunctionType
    DR = mybir.MatmulPerfMode.DoubleRow

    nc = bacc.Bacc("TRN2", target_bir_lowering=False, debug=False,
                   num_devices=NCORES)

    inH = nc.dram_tensor("inH", [D_IN, TOKENS], fp8, kind="ExternalInput")
    inL = nc.dram_tensor("inL", [D_IN, TOKENS], fp8, kind="ExternalInput")
    wT = nc.dram_tensor("wT", [D_IN, OSH], fp8, kind="ExternalInput")
    bias2d = nc.dram_tensor("bias2d", [P, OT], f32, kind="ExternalInput")
    outT = nc.dram_tensor("outT", [OSH, TOKENS], bf16, kind="ExternalOutput")
    cc_in = nc.dram_tensor("cc_in", [1, 8], f32)
    cc_out = nc.dram_tensor("cc_out", [1, 8], f32, addr_space="Shared")
    colsum_dram = nc.dram_tensor("colsum_dram", [P], f32)

    # k = ks*256 + i*128 + p (natural C-order reshape)
    inH_r = inH.ap().rearrange("(ks i p) t -> p ks i t", i=2, p=P)
    inL_r = inL.ap().rearrange("(ks i p) t -> p ks i t", i=2, p=P)
    # kk = ks*2 + i: plane-major k-tile index of 128
    wT_r = wT.ap().rearrange("(kk p) o -> p kk o", p=P)
    outT_r = outT.ap().rearrange("(o p) t -> p o t", p=P)

    # W DMA schedule in k-super (256-k) units: small first loads so the
    # first stationary tiles are ready a couple of us in.
    if KS == 8:
        WSCHED = (1, 1, 2, 2, 2)
    else:
        WSCHED = (KS,)
    NWQ = len(WSCHED)
    WQMAX = max(WSCHED)

    with tile.TileContext(nc) as tc:
        with (
            tc.tile_pool(name="const", bufs=1) as const,
            tc.tile_pool(name="wpool", bufs=1) as wpool,
            tc.tile_pool(name="wstream", bufs=2) as wstream,
            tc.tile_pool(name="small", bufs=1) as small,
            tc.tile_pool(name="inpool", bufs=60) as inpool,
            tc.tile_pool(name="outpool", bufs=5) as outpool,
            tc.tile_pool(name="pmm", bufs=8, space="PSUM") as pmm,
        ):
            bias_sb = const.tile([P, OT], f32)
            nc.gpsimd.dma_start(bias_sb[:], bias2d.ap())

            # PE clock warmup: the HAM gate holds the array at 1.2 GHz until
            # ~3.4us of sustained activity. Burn that window on throwaway
            # matmuls over a zeroed tile while the first weights stream in,
            # so the real matmuls start at 2.4 GHz.
            warm_src = const.tile([P, 256], bf16)
            nc.vector.memset(warm_src[:], 0.0)
            warm_ps = pmm.tile([P, 512], f32, tag="mm", name="warm_ps")
            NWARM = 17
            for wmm in range(NWARM):
                nc.tensor.matmul(warm_ps[0:16, 0:256], warm_src[:, 0:16],
                                 warm_src[:],
                                 start=(wmm == 0), stop=(wmm == NWARM - 1))

            # --- weight shard: sign -> fp8 (DoubleRow layout), |W| partials ---
            # Prologue: weight chunks interleaved with span-0 input loads on
            # the SP ring, so sign planes and span-0 inputs arrive in the
            # order the ks-outer span-0 loop consumes them. All DMA issues
            # precede the signs; each sign only waits on its own chunk's
            # completion semaphore.
            # Sign on ACT; |.| row-sums on DVE; no PE involvement anywhere in
            # the scale chain so the in-order PE queue is never blocked on it.
            sT = wpool.tile([P, KS, 2, OSH], fp8)
            absacc = wpool.tile([P, NWQ], f32)

            def issue_in(q, t0, ks):
                ih = inpool.tile([P, 2, SPAN], fp8, tag="in",
                                 name=f"inh{q}_{ks}")
                nc.sync.dma_start(ih[:], inH_r[:, ks, :, t0:t0 + SPAN])
                il = None
                if ks < KS - n_skip_lo:
                    il = inpool.tile([P, 2, SPAN], fp8, tag="in",
                                     name=f"inl{q}_{ks}")
                    nc.sync.dma_start(il[:], inL_r[:, ks, :, t0:t0 + SPAN])
                return (ih, il)

            wts = []
            in0 = []
            k0 = 0
            deferred_in = []
            for g, wq in enumerate(WSCHED):
                wt = wstream.tile([P, 2 * WQMAX, OSH], fp8, tag="wt",
                                  bufs=NWQ, name=f"wt{g}")
                nc.sync.dma_start(
                    wt[:, :2 * wq, :], wT_r[:, 2 * k0:2 * (k0 + wq), :]
                )
                wts.append((wt, k0, wq))
                if g >= NWQ - 2:
                    # last two weight chunks ride back-to-back so the last
                    # sign planes can start before the tail input loads
                    deferred_in.extend(range(k0, k0 + wq))
                else:
                    for ks in range(k0, k0 + wq):
                        in0.append(issue_in(0, 0, ks))
                k0 += wq
            for ks in deferred_in:
                in0.append(issue_in(0, 0, ks))
            # sign planes alternate ACT (even ks) / DVE (odd ks, as
            # multiply-clamp: the shipped fp8 weights' smallest nonzero
            # magnitude is 2^-9 * 2048-prescale units, so x*1e30 saturates
            # far past +-1 and zeros stay zero). Two engines stream planes
            # ~2x faster than one, tracking the ks-outer span-0 consumption.
            for g, (wt, k0, wq) in enumerate(wts):
                for s in range(wq):
                    ks = k0 + s
                    wsrc = wt[:, 2 * s:2 * s + 2, :]
                    if ks == 0:
                        # first plane in o-halves: the first stationary
                        # (ks0, o0) is ready ~1 us sooner
                        for h in range(2):
                            nc.scalar.activation(
                                sT[:, 0, :, h * 512:(h + 1) * 512],
                                wsrc[:, :, h * 512:(h + 1) * 512], AF.Sign)
                    elif ks % 2 == 0:
                        nc.scalar.activation(sT[:, ks, :, :], wsrc, AF.Sign)
                    else:
                        stmp = wstream.tile([P, 2, OSH], bf16, tag="stmp",
                                            bufs=2, name=f"stmp{ks}")
                        nc.vector.tensor_scalar(
                            stmp[:], wsrc, 1e30, 1.0,
                            mybir.AluOpType.mult, mybir.AluOpType.min)
                        nc.vector.tensor_scalar(
                            sT[:, ks, :, :], stmp[:], -1.0, None,
                            mybir.AluOpType.max)
            # |W| partials pinned past the sign stream (tile_wait_until):
            # they are ready before the DVE sign ops and would otherwise be
            # scheduled ahead of them; the scale isn't needed until span
            # EARLY's evictions (~50 us), so a 16 us floor costs nothing.
            with tc.tile_wait_until(0.016):
                for g, (wt, k0, wq) in enumerate(wts):
                    nc.vector.tensor_reduce(absacc[:, g:g + 1],
                                            wt[:, :2 * wq, :],
                                            axis=mybir.AxisListType.XY,
                                            op=mybir.AluOpType.add,
                                            apply_absolute_value=True)

            # --- global scale via AllReduce of the scalar partial ---
            # per-chunk |W| abs-row-sums live in absacc; fold the chunk axis
            # on DVE, then the partition axis via a DRAM bounce (the
            # partition axis can't fold into an SBUF free axis directly).
            # Scale is only needed by span EARLY's evictions (~50 us in).
            colsum = small.tile([P, 1], f32)
            nc.vector.reduce_sum(colsum[:], absacc[:],
                                 axis=mybir.AxisListType.X)
            nc.gpsimd.dma_start(colsum_dram.ap(), colsum[:, 0])
            rowt = small.tile([1, P], f32)
            nc.gpsimd.dma_start(rowt[0:1, :], colsum_dram.ap()[None, :])
            part = small.tile([1, 8], f32)
            nc.vector.memset(part[:], 0.0)
            nc.vector.reduce_sum(part[0:1, 0:1], rowt[0:1, :],
                                 axis=mybir.AxisListType.X)
            nc.gpsimd.dma_start(cc_in.ap(), part[:])
            if use_collective:
                nc.gpsimd.collective_compute(
                    "AllReduce",
                    mybir.AluOpType.add,
                    replica_groups=[list(range(NCORES))],
                    ins=[cc_in.ap()],
                    outs=[cc_out.ap()],
                )
                cc_result = cc_out
            else:
                # timing-model variant (TimelineSim can't model collectives):
                # local partial stands in for the global sum
                nc.gpsimd.dma_start(cc_out.ap(), cc_in.ap())
                cc_result = cc_out
            # broadcast the reduced scalar to all 128 partitions straight
            # from DRAM (step-0 source AP)
            scale_raw = small.tile([P, 1], f32)
            with nc.allow_non_contiguous_dma(reason="scale broadcast"):
                nc.gpsimd.dma_start(scale_raw[:, 0:1],
                                    cc_result.ap()[0:1, 0:1].to_broadcast((P, 1)))
            scale_b = small.tile([P, 1], f32)
            nc.scalar.activation(scale_b[:], scale_raw[:], AF.Copy,
                                 scale=1.0 / float(D_OUT * D_IN * W_PRESCALE))

            # --- main GEMM: outT[o, t] = sum_k sT[k, o] * (xhi+xlo)[k, t] ---
            # DoubleRow fp8: each matmul contracts 256 k (2 planes x 128
            # partitions) at 0.5 cycles per output column. hi and lo input
            # planes accumulate into the same PSUM bank; the sign stationary
            # is shared by both per (ks, o).
            spans = [(q + r * NSPAN, (q % NSPAN) * SPAN)
                     for r in range(repeat) for q in range(NSPAN)]
            nlo = KS - n_skip_lo

            def mm(ps, o, ks, pi, src, nparts):
                nc.tensor.matmul(
                    ps[:], sT[:, ks, :, o * P:(o + 1) * P], src[:],
                    start=(ks == 0 and pi == 0),
                    stop=(ks == KS - 1 and pi == nparts - 1),
                    perf_mode=DR,
                )

            def evict(stage, ps, o, early):
                if early:
                    # scale may still be in flight (the AllReduce) - plain
                    # copy frees the PSUM bank now; scale+bias folded in a
                    # DVE pass afterwards.
                    nc.scalar.activation(stage[:, o, :], ps[:], AF.Copy)
                    nc.vector.tensor_scalar(
                        stage[:, o, :], stage[:, o, :],
                        scale_b[:, 0:1], bias_sb[:, o:o + 1],
                        mybir.AluOpType.mult, mybir.AluOpType.add)
                else:
                    nc.scalar.activation(
                        stage[:, o, :], ps[:], AF.Identity,
                        bias=bias_sb[:, o:o + 1], scale=scale_b[:, 0:1])

            for q, t0 in spans:
                early = q < EARLY
                if q == 0:
                    inq = in0
                else:
                    inq = [issue_in(q, t0, ks) for ks in range(KS)]
                stage = outpool.tile([P, OT, SPAN], bf16, tag="stage",
                                     name=f"st{q}")
                psums = [pmm.tile([P, SPAN], f32, tag="mm",
                                  name=f"pp{q}_{o}") for o in range(OT)]
                if q < 2:
                    # ks-outer: consume each sign plane / input tile as it's
                    # produced; all 8 PSUM banks accumulate simultaneously.
                    # At the last ks, finish + evict per o so banks free for
                    # the next span as the ACT engine catches up.
                    for ks in range(KS - 1):
                        for o in range(OT):
                            for pi, src in enumerate(
                                    s for s in inq[ks] if s is not None):
                                mm(psums[o], o, ks, pi, src,
                                   2 if ks < nlo else 1)
                    ks = KS - 1
                    for o in range(OT):
                        for pi, src in enumerate(
                                s for s in inq[ks] if s is not None):
                            mm(psums[o], o, ks, pi, src, 2 if ks < nlo else 1)
                        evict(stage, psums[o], o, early)
                else:
                    last = q == spans[-1][0]
                    for o in range(OT):
                        for ks in range(KS):
                            for pi, src in enumerate(
                                    s for s in inq[ks] if s is not None):
                                mm(psums[o], o, ks, pi, src,
                                   2 if ks < nlo else 1)
                        evict(stage, psums[o], o, early)
                        if last:
                            # per-o stores right behind each eviction, on
                            # the SP ring - input loads are done by now, so
                            # SP is idle and the ACT queue keeps evicting:
                            # the drain tail is one small DMA
                            nc.sync.dma_start(outT_r[:, o, t0:t0 + SPAN],
                                              stage[:, o, :])
                    if last:
                        continue
                # batched stores per span half on the SWDGE queue (two
                # ~1.5 us device slices interleave with input loads better
                # than one 3 us one)
                h = OT // 2
                nc.gpsimd.dma_start(outT_r[:, 0:h, t0:t0 + SPAN],
                                    stage[:, 0:h, :])
                nc.gpsimd.dma_start(outT_r[:, h:, t0:t0 + SPAN],
                                    stage[:, h:, :])

    if dedup_ldw:
        _dedup_ldweights(nc, mybir)
    nc.compile()
    return nc


def _dedup_ldweights(nc, mybir):
    """Drop consecutive InstLdweights that reload the exact same stationary
    AP with only matmuls in between. Tile emits one weight load per matmul
    even when several matmuls share a stationary; the following
    non-self-loading matmuls keep using the already-loaded array state.
    Only waitless/updateless loads are removed."""
    removed = 0
    for bb in nc.m.functions[0].blocks:
        il = bb.instructions
        kept = []
        prev_sig = None
        for i in il:
            if isinstance(i, mybir.InstLdweights):
                sig = str(i.ins[0])
                if (sig == prev_sig and not i.has_wait()
                        and not i.has_update()):
                    nc.inst_map.pop(i.name, None)
                    removed += 1
                    continue
                prev_sig = sig
            elif isinstance(i, mybir.InstMatmult):
                pass
            elif getattr(i, "engine", None) == mybir.EngineType.PE:
                prev_sig = None
            kept.append(i)
        il[:] = kept


def _get_nc():
    if "nc" not in _NC_CACHE:
        _NC_CACHE["nc"] = _build_nc()
    return _NC_CACHE["nc"]


def _make_in_maps(input, weight, bias):
    inT = np.ascontiguousarray(input.T)
    inT_hi = inT.astype(ml_dtypes.float8_e4m3)
    inT_lo = (inT - inT_hi.astype(np.float32)).astype(ml_dtypes.float8_e4m3)
    wT_full = weight.T  # [D_IN, D_OUT] view
    in_maps = []
    for j in range(NCORES):
        bsh = bias[j * OSH:(j + 1) * OSH]
        in_maps.append({
            "inH": inT_hi,
            "inL": inT_lo,
            "wT": (np.ascontiguousarray(wT_full[:, j * OSH:(j + 1) * OSH])
                   * np.float32(W_PRESCALE)).astype(ml_dtypes.float8_e4m3),
            "bias2d": np.ascontiguousarray(
                bsh.reshape(OT, P).T, dtype=np.float32),
        })
    return in_maps


def run(input, weight, bias, trace=False, **spmd_kwargs):
    from concourse.bass_utils import run_bass_kernel_spmd

    nc = _get_nc()
    in_maps = _make_in_maps(np.asarray(input, dtype=np.float32),
                            np.asarray(weight, dtype=np.float32),
                            np.asarray(bias, dtype=np.float32))
    res = run_bass_kernel_spmd(nc, in_maps, core_ids=list(range(NCORES)),
                               trace=trace, **spmd_kwargs)
    outT = np.concatenate([r["outT"] for r in res.results], axis=0)
    out = np.ascontiguousarray(outT.T.astype(np.float32))
    return out, res


def kernel(input, weight, bias):
    out, _ = run(input, weight, bias, trace=False)
    return out


# revision 39
# speedup vs baseline: 1.0916x; 1.0091x over previous
"""BitLinear (BitNet-style) kernel for 8 Trainium2 NeuronCores.

Computes: out = input @ (sign(W) * mean(|W|)).T + bias
  input [8192, 2048] f32, W [8192, 2048] f32, bias [8192] f32 -> out [8192, 8192] f32

Sharding: column-parallel over out_features. Core j owns W rows
[j*1024, (j+1)*1024). Each core computes sign() on its shard (split
between the scalar and vector engines) and a local |W| partial sum
(vector engine reduce with absolute value); partial sums are
AllReduce'd across the 8 cores so the scale is the global abs-mean.

GEMM precision/speed: the PE's fp8 DoubleRow mode packs two k-planes per
matmul (stationary [128,2,M], moving [128,2,N]) and streams at 0.5
cycles per output column - 2x the bf16 column rate with twice the K per
step. sign(W) is exactly representable in fp8e4, and the input is fed as
an exact-ish hi+lo pair: x_hi = fp8(x), x_lo = fp8(x - x_hi), both
multiplied against the same sign stationary into the same PSUM
accumulation, recovering ~11 mantissa bits. The last N_SKIP_LO=4 of the
8 k-super-steps skip the lo correction (each skipped step adds
sqrt(1/8)*2.66e-2 in quadrature; measured 1.905e-2 end-to-end vs the
2e-2 gate - the rel-err is a population statistic over 67M outputs, so
it is stable to ~1e-4 relative) and save 1/16 of the PE time each.

Weights ship as fp8e4 of (W.T * 2048): sign is preserved (only |w| <
4.8e-7 quantizes to 0 - 119 of 16.7M elements, ~1e-3 quadrature error)
and the |W| partial sums come out scaled by 2048, folded into the final
scale constant. This halves weight DMA vs bf16 and gets the first
stationary ready sooner.

scale (fp32) and bias (fp32) are fused into the PSUM->SBUF eviction:
out = psum * scale + bias, written as bf16 (~1e-3 rounding, halves
store traffic); the host concatenates, transposes and upcasts.

Layout: host ships the input as two fp8 planes inH/inL of shape
[D_IN, TOKENS] (k-major). k is split (ks, i, p) = (super-step, DoubleRow
plane, partition): k = ks*256 + i*128 + p, a natural C-order reshape on
both operands so no host shuffling beyond the transpose.

Perf notes (cost-model profiled, 176.1 us vs 450.1 us bf16 baseline):
- 1536 DoubleRow matmuls of [K=256]x[M=128 o]x[N=512 t] at ~107 ns each
  (~164 us PE busy); the bf16 kernel's floor was ~438 us.
- 16 uniform 512-token spans: input arrives in 364 ns quanta; steady
  DMA per span (~7 us) is well under PE per span (~10 us). Total DMA
  ~125 us vs the 360 B/ns aggregate ring rate.
- Sign planes are produced by TWO engines in parallel - ACT (AF.Sign,
  even ks, the first plane in o-halves) and DVE (odd ks, as a
  multiply-clamp: the shipped fp8 weights' smallest nonzero magnitude
  makes x*1e30 saturate far past +-1, then min/max clamp; exact 0 stays
  0). One engine alone (1.9 us/plane) cannot keep up with the ks-outer
  span-0/1 consumption and costs ~3 us of PE idle.
- The |W| partial reduces are pinned past the sign stream with
  tc.tile_wait_until: the Tile scheduler orders by dependency readiness,
  and the early-ready reduces would otherwise run before the DVE sign
  ops (measured, not hypothetical). The scale isn't needed until span
  EARLY's evictions, so a 16 us floor costs nothing.
- Prologue interleaves weight chunks with span-0 input loads on the SP
  ring (last two chunks back-to-back so the tail planes sign early);
  spans 0-1 run ks-outer (all 8 PSUM banks open) consuming each plane
  as it is produced; PE warmup matmuls cover the first ~4.7 us.
- Per-span staging tile [128, 8, 512] bf16 (outpool bufs=5 rides out
  the early-span store delay) and batched SWDGE half-span stores:
  stores never sit on the ACT/SP sequencers where they would
  head-of-line block evictions (PSUM-bank back-pressure -> PE stall) or
  input loads. The last span stores per-o on the then-idle SP ring so
  the drain tail is one small DMA. inpool bufs=60 gives ~5 spans of
  input prefetch credit so store bursts on the shared DMA device never
  starve the PE.
- First 3 spans evict with a plain copy and fold scale+bias in a second
  DVE pass, so nothing stalls on the AllReduce latency.
"""

import sys

for _p in ("/opt/trn_rl_repo",):
    if _p not in sys.path:
        sys.path.append(_p)

import ml_dtypes
import numpy as np

TOKENS = 8192
D_IN = 2048
D_OUT = 8192
NCORES = 8
OSH = D_OUT // NCORES  # 1024 out features per core
P = 128
KS = D_IN // (2 * P)   # 8 k-super-tiles of 256 (two DoubleRow planes)
OT = OSH // P          # 8 o-tiles per core
SPAN = 512
NSPAN = TOKENS // SPAN
EARLY = 3              # spans evicted before the scale is known
N_SKIP_LO = 4          # k-super-steps (from the end) without lo correction
W_PRESCALE = 2048.0    # host premultiplier so fp8(W.T) keeps tiny signs

_NC_CACHE = {}


def _build_nc(use_collective=True, repeat=1, dedup_ldw=True,
              n_skip_lo=N_SKIP_LO):
    import concourse.mybir as mybir
    import concourse.tile as tile
    from concourse import bacc

    f32 = mybir.dt.float32
    bf16 = mybir.dt.bfloat16
    fp8 = mybir.dt.float8e4
    AF = mybir.ActivationFunctionType
    DR = mybir.MatmulPerfMode.DoubleRow

    nc = bacc.Bacc("TRN2", target_bir_lowering=False, debug=False,
                   num_devices=NCORES)

    inH = nc.dram_tensor("inH", [D_IN, TOKENS], fp8, kind="ExternalInput")
    inL = nc.dram_tensor("inL", [D_IN, TOKENS], fp8, kind="ExternalInput")
    wT = nc.dram_tensor("wT", [D_IN, OSH], fp8, kind="ExternalInput")
    bias2d = nc.dram_tensor("bias2d", [P, OT], f32, kind="ExternalInput")
    outT = nc.dram_tensor("outT", [OSH, TOKENS], bf16, kind="ExternalOutput")
    cc_in = nc.dram_tensor("cc_in", [1, 8], f32)
    cc_out = nc.dram_tensor("cc_out", [1, 8], f32, addr_space="Shared")
    colsum_dram = nc.dram_tensor("colsum_dram", [P], f32)

    # k = ks*256 + i*128 + p (natural C-order reshape)
    inH_r = inH.ap().rearrange("(ks i p) t -> p ks i t", i=2, p=P)
    inL_r = inL.ap().rearrange("(ks i p) t -> p ks i t", i=2, p=P)
    # kk = ks*2 + i: plane-major k-tile index of 128
    wT_r = wT.ap().rearrange("(kk p) o -> p kk o", p=P)
    outT_r = outT.ap().rearrange("(o p) t -> p o t", p=P)

    # W DMA schedule in k-super (256-k) units: small first loads so the
    # first stationary tiles are ready a couple of us in.
    if KS == 8:
        WSCHED = (1, 1, 2, 2, 2)
    else:
        WSCHED = (KS,)
    NWQ = len(WSCHED)
    WQMAX = max(WSCHED)

    with tile.TileContext(nc) as tc:
        with (
            tc.tile_pool(name="const", bufs=1) as const,
            tc.tile_pool(name="wpool", bufs=1) as wpool,
            tc.tile_pool(name="wstream", bufs=2) as wstream,
            tc.tile_pool(name="small", bufs=1) as small,
            tc.tile_pool(name="inpool", bufs=60) as inpool,
            tc.tile_pool(name="outpool", bufs=5) as outpool,
            tc.tile_pool(name="pmm", bufs=8, space="PSUM") as pmm,
        ):
            # PE clock warmup: the HAM gate holds the array at 1.2 GHz until
            # ~3.4us of sustained activity. Burn that window on throwaway
            # matmuls over a zeroed tile while the first weights stream in,
            # so the real matmuls start at 2.4 GHz. The memset rides Pool
            # ahead of the bias load so the PE starts as early as possible.
            warm_src = const.tile([P, 256], bf16)
            nc.gpsimd.memset(warm_src[:], 0.0)
            bias_sb = const.tile([P, OT], f32)
            nc.gpsimd.dma_start(bias_sb[:], bias2d.ap())
            warm_ps = pmm.tile([P, 512], f32, tag="mm", name="warm_ps")
            NWARM = 17
            for wmm in range(NWARM):
                nc.tensor.matmul(warm_ps[0:16, 0:256], warm_src[:, 0:16],
                                 warm_src[:],
                                 start=(wmm == 0), stop=(wmm == NWARM - 1))

            # --- weight shard: sign -> fp8 (DoubleRow layout), |W| partials ---
            # Prologue: weight chunks interleaved with span-0 input loads on
            # the SP ring, so sign planes and span-0 inputs arrive in the
            # order the ks-outer span-0 loop consumes them. All DMA issues
            # precede the signs; each sign only waits on its own chunk's
            # completion semaphore.
            # Sign on ACT; |.| row-sums on DVE; no PE involvement anywhere in
            # the scale chain so the in-order PE queue is never blocked on it.
            sT = wpool.tile([P, KS, 2, OSH], fp8)
            absacc = wpool.tile([P, NWQ], f32)

            def issue_in(q, t0, ks):
                ih = inpool.tile([P, 2, SPAN], fp8, tag="in",
                                 name=f"inh{q}_{ks}")
                nc.sync.dma_start(ih[:], inH_r[:, ks, :, t0:t0 + SPAN])
                il = None
                if ks < KS - n_skip_lo:
                    il = inpool.tile([P, 2, SPAN], fp8, tag="in",
                                     name=f"inl{q}_{ks}")
                    nc.sync.dma_start(il[:], inL_r[:, ks, :, t0:t0 + SPAN])
                return (ih, il)

            wts = []
            in0 = []
            k0 = 0
            deferred_in = []
            for g, wq in enumerate(WSCHED):
                wt = wstream.tile([P, 2 * WQMAX, OSH], fp8, tag="wt",
                                  bufs=NWQ, name=f"wt{g}")
                nc.sync.dma_start(
                    wt[:, :2 * wq, :], wT_r[:, 2 * k0:2 * (k0 + wq), :]
                )
                wts.append((wt, k0, wq))
                if g >= NWQ - 2:
                    # last two weight chunks ride back-to-back so the last
                    # sign planes can start before the tail input loads
                    deferred_in.extend(range(k0, k0 + wq))
                else:
                    for ks in range(k0, k0 + wq):
                        in0.append(issue_in(0, 0, ks))
                k0 += wq
            for ks in deferred_in:
                in0.append(issue_in(0, 0, ks))
            # sign planes alternate ACT (even ks) / DVE (odd ks, as
            # multiply-clamp: the shipped fp8 weights' smallest nonzero
            # magnitude is 2^-9 * 2048-prescale units, so x*1e30 saturates
            # far past +-1 and zeros stay zero). Two engines stream planes
            # ~2x faster than one, tracking the ks-outer span-0 consumption.
            for g, (wt, k0, wq) in enumerate(wts):
                for s in range(wq):
                    ks = k0 + s
                    wsrc = wt[:, 2 * s:2 * s + 2, :]
                    if ks == 0:
                        # first plane in o-halves: the first stationary
                        # (ks0, o0) is ready ~1 us sooner
                        for h in range(2):
                            nc.scalar.activation(
                                sT[:, 0, :, h * 512:(h + 1) * 512],
                                wsrc[:, :, h * 512:(h + 1) * 512], AF.Sign)
                    elif ks % 2 == 0:
                        nc.scalar.activation(sT[:, ks, :, :], wsrc, AF.Sign)
                    else:
                        stmp = wstream.tile([P, 2, OSH], bf16, tag="stmp",
                                            bufs=2, name=f"stmp{ks}")
                        nc.vector.tensor_scalar(
                            stmp[:], wsrc, 1e30, 1.0,
                            mybir.AluOpType.mult, mybir.AluOpType.min)
                        nc.vector.tensor_scalar(
                            sT[:, ks, :, :], stmp[:], -1.0, None,
                            mybir.AluOpType.max)
            # |W| partials pinned past the sign stream (tile_wait_until):
            # they are ready before the DVE sign ops and would otherwise be
            # scheduled ahead of them; the scale isn't needed until span
            # EARLY's evictions (~50 us), so a 16 us floor costs nothing.
            with tc.tile_wait_until(0.016):
                for g, (wt, k0, wq) in enumerate(wts):
                    nc.vector.tensor_reduce(absacc[:, g:g + 1],
                                            wt[:, :2 * wq, :],
                                            axis=mybir.AxisListType.XY,
                                            op=mybir.AluOpType.add,
                                            apply_absolute_value=True)

            # --- global scale via AllReduce of the scalar partial ---
            # per-chunk |W| abs-row-sums live in absacc; fold the chunk axis
            # on DVE, then the partition axis via a DRAM bounce (the
            # partition axis can't fold into an SBUF free axis directly).
            # Scale is only needed by span EARLY's evictions (~50 us in).
            colsum = small.tile([P, 1], f32)
            nc.vector.reduce_sum(colsum[:], absacc[:],
                                 axis=mybir.AxisListType.X)
            nc.gpsimd.dma_start(colsum_dram.ap(), colsum[:, 0])
            rowt = small.tile([1, P], f32)
            nc.gpsimd.dma_start(rowt[0:1, :], colsum_dram.ap()[None, :])
            part = small.tile([1, 8], f32)
            nc.vector.memset(part[:], 0.0)
            nc.vector.reduce_sum(part[0:1, 0:1], rowt[0:1, :],
                                 axis=mybir.AxisListType.X)
            nc.gpsimd.dma_start(cc_in.ap(), part[:])
            if use_collective:
                nc.gpsimd.collective_compute(
                    "AllReduce",
                    mybir.AluOpType.add,
                    replica_groups=[list(range(NCORES))],
                    ins=[cc_in.ap()],
                    outs=[cc_out.ap()],
                )
                cc_result = cc_out
            else:
                # timing-model variant (TimelineSim can't model collectives):
                # local partial stands in for the global sum
                nc.gpsimd.dma_start(cc_out.ap(), cc_in.ap())
                cc_result = cc_out
            # broadcast the reduced scalar to all 128 partitions straight
            # from DRAM (step-0 source AP)
            scale_raw = small.tile([P, 1], f32)
            with nc.allow_non_contiguous_dma(reason="scale broadcast"):
                nc.gpsimd.dma_start(scale_raw[:, 0:1],
                                    cc_result.ap()[0:1, 0:1].to_broadcast((P, 1)))
            scale_b = small.tile([P, 1], f32)
            nc.scalar.activation(scale_b[:], scale_raw[:], AF.Copy,
                                 scale=1.0 / float(D_OUT * D_IN * W_PRESCALE))

            # --- main GEMM: outT[o, t] = sum_k sT[k, o] * (xhi+xlo)[k, t] ---
            # DoubleRow fp8: each matmul contracts 256 k (2 planes x 128
            # partitions) at 0.5 cycles per output column. hi and lo input
            # planes accumulate into the same PSUM bank; the sign stationary
            # is shared by both per (ks, o).
            spans = [(q + r * NSPAN, (q % NSPAN) * SPAN)
                     for r in range(repeat) for q in range(NSPAN)]
            nlo = KS - n_skip_lo

            def mm(ps, o, ks, pi, src, nparts):
                nc.tensor.matmul(
                    ps[:], sT[:, ks, :, o * P:(o + 1) * P], src[:],
                    start=(ks == 0 and pi == 0),
                    stop=(ks == KS - 1 and pi == nparts - 1),
                    perf_mode=DR,
                )

            def evict(stage, ps, o, early):
                if early:
                    # scale may still be in flight (the AllReduce) - plain
                    # copy frees the PSUM bank now; scale+bias folded in a
                    # DVE pass afterwards.
                    nc.scalar.activation(stage[:, o, :], ps[:], AF.Copy)
                    nc.vector.tensor_scalar(
                        stage[:, o, :], stage[:, o, :],
                        scale_b[:, 0:1], bias_sb[:, o:o + 1],
                        mybir.AluOpType.mult, mybir.AluOpType.add)
                else:
                    nc.scalar.activation(
                        stage[:, o, :], ps[:], AF.Identity,
                        bias=bias_sb[:, o:o + 1], scale=scale_b[:, 0:1])

            for q, t0 in spans:
                early = q < EARLY
                if q == 0:
                    inq = in0
                else:
                    inq = [issue_in(q, t0, ks) for ks in range(KS)]
                stage = outpool.tile([P, OT, SPAN], bf16, tag="stage",
                                     name=f"st{q}")
                psums = [pmm.tile([P, SPAN], f32, tag="mm",
                                  name=f"pp{q}_{o}") for o in range(OT)]
                if q < 2:
                    # ks-outer: consume each sign plane / input tile as it's
                    # produced; all 8 PSUM banks accumulate simultaneously.
                    # At the last ks, finish + evict per o so banks free for
                    # the next span as the ACT engine catches up.
                    for ks in range(KS - 1):
                        for o in range(OT):
                            for pi, src in enumerate(
                                    s for s in inq[ks] if s is not None):
                                mm(psums[o], o, ks, pi, src,
                                   2 if ks < nlo else 1)
                    ks = KS - 1
                    for o in range(OT):
                        for pi, src in enumerate(
                                s for s in inq[ks] if s is not None):
                            mm(psums[o], o, ks, pi, src, 2 if ks < nlo else 1)
                        evict(stage, psums[o], o, early)
                else:
                    last = q == spans[-1][0]
                    for o in range(OT):
                        for ks in range(KS):
                            for pi, src in enumerate(
                                    s for s in inq[ks] if s is not None):
                                mm(psums[o], o, ks, pi, src,
                                   2 if ks < nlo else 1)
                        evict(stage, psums[o], o, early)
                        if last:
                            # per-o stores right behind each eviction, on
                            # the SP ring - input loads are done by now, so
                            # SP is idle and the ACT queue keeps evicting:
                            # the drain tail is one small DMA
                            nc.sync.dma_start(outT_r[:, o, t0:t0 + SPAN],
                                              stage[:, o, :])
                    if last:
                        continue
                # batched stores per span half on the SWDGE queue (two
                # ~1.5 us device slices interleave with input loads better
                # than one 3 us one)
                h = OT // 2
                nc.gpsimd.dma_start(outT_r[:, 0:h, t0:t0 + SPAN],
                                    stage[:, 0:h, :])
                nc.gpsimd.dma_start(outT_r[:, h:, t0:t0 + SPAN],
                                    stage[:, h:, :])

    if dedup_ldw:
        _dedup_ldweights(nc, mybir)
    nc.compile()
    return nc


def _dedup_ldweights(nc, mybir):
    """Drop consecutive InstLdweights that reload the exact same stationary
    AP with only matmuls in between. Tile emits one weight load per matmul
    even when several matmuls share a stationary; the following
    non-self-loading matmuls keep using the already-loaded array state.
    Only waitless/updateless loads are removed."""
    removed = 0
    for bb in nc.m.functions[0].blocks:
        il = bb.instructions
        kept = []
        prev_sig = None
        for i in il:
            if isinstance(i, mybir.InstLdweights):
                sig = str(i.ins[0])
                if (sig == prev_sig and not i.has_wait()
                        and not i.has_update()):
                    nc.inst_map.pop(i.name, None)
                    removed += 1
                    continue
                prev_sig = sig
            elif isinstance(i, mybir.InstMatmult):
                pass
            elif getattr(i, "engine", None) == mybir.EngineType.PE:
                prev_sig = None
            kept.append(i)
        il[:] = kept


def _get_nc():
    if "nc" not in _NC_CACHE:
        _NC_CACHE["nc"] = _build_nc()
    return _NC_CACHE["nc"]


def _make_in_maps(input, weight, bias):
    inT = np.ascontiguousarray(input.T)
    inT_hi = inT.astype(ml_dtypes.float8_e4m3)
    inT_lo = (inT - inT_hi.astype(np.float32)).astype(ml_dtypes.float8_e4m3)
    wT_full = weight.T  # [D_IN, D_OUT] view
    in_maps = []
    for j in range(NCORES):
        bsh = bias[j * OSH:(j + 1) * OSH]
        in_maps.append({
            "inH": inT_hi,
            "inL": inT_lo,
            "wT": (np.ascontiguousarray(wT_full[:, j * OSH:(j + 1) * OSH])
                   * np.float32(W_PRESCALE)).astype(ml_dtypes.float8_e4m3),
            "bias2d": np.ascontiguousarray(
                bsh.reshape(OT, P).T, dtype=np.float32),
        })
    return in_maps


def run(input, weight, bias, trace=False, **spmd_kwargs):
    from concourse.bass_utils import run_bass_kernel_spmd

    nc = _get_nc()
    in_maps = _make_in_maps(np.asarray(input, dtype=np.float32),
                            np.asarray(weight, dtype=np.float32),
                            np.asarray(bias, dtype=np.float32))
    res = run_bass_kernel_spmd(nc, in_maps, core_ids=list(range(NCORES)),
                               trace=trace, **spmd_kwargs)
    outT = np.concatenate([r["outT"] for r in res.results], axis=0)
    out = np.ascontiguousarray(outT.T.astype(np.float32))
    return out, res


def kernel(input, weight, bias):
    out, _ = run(input, weight, bias, trace=False)
    return out


# revision 44
# speedup vs baseline: 1.0919x; 1.0002x over previous
"""BitLinear (BitNet-style) kernel for 8 Trainium2 NeuronCores.

Computes: out = input @ (sign(W) * mean(|W|)).T + bias
  input [8192, 2048] f32, W [8192, 2048] f32, bias [8192] f32 -> out [8192, 8192] f32

Sharding: column-parallel over out_features. Core j owns W rows
[j*1024, (j+1)*1024). Each core computes sign() on its shard (split
between the scalar and vector engines) and a local |W| partial sum
(vector engine reduce with absolute value); partial sums are
AllReduce'd across the 8 cores so the scale is the global abs-mean.

GEMM precision/speed: the PE's fp8 DoubleRow mode packs two k-planes per
matmul (stationary [128,2,M], moving [128,2,N]) and streams at 0.5
cycles per output column - 2x the bf16 column rate with twice the K per
step. sign(W) is exactly representable in fp8e4, and the input is fed as
an exact-ish hi+lo pair: x_hi = fp8(x), x_lo = fp8(x - x_hi), both
multiplied against the same sign stationary into the same PSUM
accumulation, recovering ~11 mantissa bits. The last N_SKIP_LO=4 of the
8 k-super-steps skip the lo correction (each skipped step adds
sqrt(1/8)*2.66e-2 in quadrature; measured 1.905e-2 end-to-end vs the
2e-2 gate - the rel-err is a population statistic over 67M outputs, so
it is stable to ~1e-4 relative) and save 1/16 of the PE time each.

Weights ship as fp8e4 of (W.T * 2048): sign is preserved (only |w| <
4.8e-7 quantizes to 0 - 119 of 16.7M elements, ~1e-3 quadrature error)
and the |W| partial sums come out scaled by 2048, folded into the final
scale constant. This halves weight DMA vs bf16 and gets the first
stationary ready sooner.

scale (fp32) and bias (fp32) are fused into the PSUM->SBUF eviction:
out = psum * scale + bias, written as bf16 (~1e-3 rounding, halves
store traffic); the host concatenates, transposes and upcasts.

Layout: host ships the input as two fp8 planes inH/inL of shape
[D_IN, TOKENS] (k-major). k is split (ks, i, p) = (super-step, DoubleRow
plane, partition): k = ks*256 + i*128 + p, a natural C-order reshape on
both operands so no host shuffling beyond the transpose.

Perf notes (cost-model profiled, 176.1 us vs 450.1 us bf16 baseline):
- 1536 DoubleRow matmuls of [K=256]x[M=128 o]x[N=512 t] at ~107 ns each
  (~164 us PE busy); the bf16 kernel's floor was ~438 us.
- 16 uniform 512-token spans: input arrives in 364 ns quanta; steady
  DMA per span (~7 us) is well under PE per span (~10 us). Total DMA
  ~125 us vs the 360 B/ns aggregate ring rate.
- Sign planes are produced by TWO engines in parallel - ACT (AF.Sign,
  even ks, the first plane in o-halves) and DVE (odd ks, as a
  multiply-clamp: the shipped fp8 weights' smallest nonzero magnitude
  makes x*1e30 saturate far past +-1, then min/max clamp; exact 0 stays
  0). One engine alone (1.9 us/plane) cannot keep up with the ks-outer
  span-0/1 consumption and costs ~3 us of PE idle.
- The |W| partial reduces are pinned past the sign stream with
  tc.tile_wait_until: the Tile scheduler orders by dependency readiness,
  and the early-ready reduces would otherwise run before the DVE sign
  ops (measured, not hypothetical). The scale isn't needed until span
  EARLY's evictions, so a 16 us floor costs nothing.
- Prologue interleaves weight chunks with span-0 input loads on the SP
  ring (last two chunks back-to-back so the tail planes sign early);
  spans 0-1 run ks-outer (all 8 PSUM banks open) consuming each plane
  as it is produced; PE warmup matmuls cover the first ~4.7 us.
- Per-span staging tile [128, 8, 512] bf16 (outpool bufs=5 rides out
  the early-span store delay) and batched SWDGE half-span stores:
  stores never sit on the ACT/SP sequencers where they would
  head-of-line block evictions (PSUM-bank back-pressure -> PE stall) or
  input loads. The last span stores per-o on the then-idle SP ring so
  the drain tail is one small DMA. inpool bufs=60 gives ~5 spans of
  input prefetch credit so store bursts on the shared DMA device never
  starve the PE.
- First 3 spans evict with a plain copy and fold scale+bias in a second
  DVE pass, so nothing stalls on the AllReduce latency.
"""

import sys

for _p in ("/opt/trn_rl_repo",):
    if _p not in sys.path:
        sys.path.append(_p)

import ml_dtypes
import numpy as np

TOKENS = 8192
D_IN = 2048
D_OUT = 8192
NCORES = 8
OSH = D_OUT // NCORES  # 1024 out features per core
P = 128
KS = D_IN // (2 * P)   # 8 k-super-tiles of 256 (two DoubleRow planes)
OT = OSH // P          # 8 o-tiles per core
SPAN = 512
NSPAN = TOKENS // SPAN
EARLY = 3              # spans evicted before the scale is known
N_SKIP_LO = 4          # k-super-steps (from the end) without lo correction
W_PRESCALE = 2048.0    # host premultiplier so fp8(W.T) keeps tiny signs

_NC_CACHE = {}


def _build_nc(use_collective=True, repeat=1, dedup_ldw=True,
              n_skip_lo=N_SKIP_LO):
    import concourse.mybir as mybir
    import concourse.tile as tile
    from concourse import bacc

    f32 = mybir.dt.float32
    bf16 = mybir.dt.bfloat16
    fp8 = mybir.dt.float8e4
    AF = mybir.ActivationFunctionType
    DR = mybir.MatmulPerfMode.DoubleRow

    nc = bacc.Bacc("TRN2", target_bir_lowering=False, debug=False,
                   num_devices=NCORES)

    inH = nc.dram_tensor("inH", [D_IN, TOKENS], fp8, kind="ExternalInput")
    inL = nc.dram_tensor("inL", [D_IN, TOKENS], fp8, kind="ExternalInput")
    wT = nc.dram_tensor("wT", [D_IN, OSH], fp8, kind="ExternalInput")
    bias2d = nc.dram_tensor("bias2d", [P, OT], f32, kind="ExternalInput")
    outT = nc.dram_tensor("outT", [OSH, TOKENS], bf16, kind="ExternalOutput")
    cc_in = nc.dram_tensor("cc_in", [1, 8], f32)
    cc_out = nc.dram_tensor("cc_out", [1, 8], f32, addr_space="Shared")
    colsum_dram = nc.dram_tensor("colsum_dram", [P], f32)

    # k = ks*256 + i*128 + p (natural C-order reshape)
    inH_r = inH.ap().rearrange("(ks i p) t -> p ks i t", i=2, p=P)
    inL_r = inL.ap().rearrange("(ks i p) t -> p ks i t", i=2, p=P)
    # kk = ks*2 + i: plane-major k-tile index of 128
    wT_r = wT.ap().rearrange("(kk p) o -> p kk o", p=P)
    outT_r = outT.ap().rearrange("(o p) t -> p o t", p=P)

    # W DMA schedule in k-super (256-k) units: small first loads so the
    # first stationary tiles are ready a couple of us in.
    if KS == 8:
        WSCHED = (1, 1, 2, 2, 2)
    else:
        WSCHED = (KS,)
    NWQ = len(WSCHED)
    WQMAX = max(WSCHED)

    with tile.TileContext(nc) as tc:
        with (
            tc.tile_pool(name="const", bufs=1) as const,
            tc.tile_pool(name="wpool", bufs=1) as wpool,
            tc.tile_pool(name="wstream", bufs=2) as wstream,
            tc.tile_pool(name="small", bufs=1) as small,
            tc.tile_pool(name="inpool", bufs=60) as inpool,
            tc.tile_pool(name="outpool", bufs=5) as outpool,
            tc.tile_pool(name="pmm", bufs=8, space="PSUM") as pmm,
        ):
            # PE clock warmup: the HAM gate holds the array at 1.2 GHz until
            # ~3.4us of sustained activity. Burn that window on throwaway
            # matmuls over a zeroed tile while the first weights stream in,
            # so the real matmuls start at 2.4 GHz. The memset rides Pool
            # ahead of the bias load so the PE starts as early as possible.
            warm_src = const.tile([P, 256], bf16)
            nc.gpsimd.memset(warm_src[:], 0.0)
            bias_sb = const.tile([P, OT], f32)
            nc.gpsimd.dma_start(bias_sb[:], bias2d.ap())
            warm_ps = pmm.tile([P, 512], f32, tag="mm", name="warm_ps")
            NWARM = 17
            for wmm in range(NWARM):
                nc.tensor.matmul(warm_ps[0:16, 0:256], warm_src[:, 0:16],
                                 warm_src[:],
                                 start=(wmm == 0), stop=(wmm == NWARM - 1))

            # --- weight shard: sign -> fp8 (DoubleRow layout), |W| partials ---
            # Prologue: weight chunks interleaved with span-0 input loads on
            # the SP ring, so sign planes and span-0 inputs arrive in the
            # order the ks-outer span-0 loop consumes them. All DMA issues
            # precede the signs; each sign only waits on its own chunk's
            # completion semaphore.
            # Sign on ACT; |.| row-sums on DVE; no PE involvement anywhere in
            # the scale chain so the in-order PE queue is never blocked on it.
            sT = wpool.tile([P, KS, 2, OSH], fp8)
            absacc = wpool.tile([P, NWQ], f32)

            def issue_in(q, t0, ks):
                ih = inpool.tile([P, 2, SPAN], fp8, tag="in",
                                 name=f"inh{q}_{ks}")
                nc.sync.dma_start(ih[:], inH_r[:, ks, :, t0:t0 + SPAN])
                il = None
                if ks < KS - n_skip_lo:
                    il = inpool.tile([P, 2, SPAN], fp8, tag="in",
                                     name=f"inl{q}_{ks}")
                    nc.sync.dma_start(il[:], inL_r[:, ks, :, t0:t0 + SPAN])
                return (ih, il)

            wts = []
            in0 = []
            k0 = 0
            deferred_in = []
            for g, wq in enumerate(WSCHED):
                wt = wstream.tile([P, 2 * WQMAX, OSH], fp8, tag="wt",
                                  bufs=NWQ, name=f"wt{g}")
                # first chunk on the ACT ring: it comes up slightly sooner
                # and SP's first slot goes to the first input tile instead
                weng = nc.scalar if g == 0 else nc.sync
                weng.dma_start(
                    wt[:, :2 * wq, :], wT_r[:, 2 * k0:2 * (k0 + wq), :]
                )
                wts.append((wt, k0, wq))
                if g >= NWQ - 2:
                    # last two weight chunks ride back-to-back so the last
                    # sign planes can start before the tail input loads
                    deferred_in.extend(range(k0, k0 + wq))
                else:
                    for ks in range(k0, k0 + wq):
                        in0.append(issue_in(0, 0, ks))
                k0 += wq
            for ks in deferred_in:
                in0.append(issue_in(0, 0, ks))
            # every sign plane is produced by BOTH engines in halves:
            # ACT signs o[0:512] (AF.Sign), DVE signs o[512:1024] as a
            # multiply-clamp ((x*1e30) min 1, then max -1 - exact for fp8
            # sources since the smallest nonzero magnitude is 2^-9 in
            # prescaled units; zeros stay zero). Together they finish each
            # plane ~2x faster than either engine alone, tracking the
            # ks-outer span-0/1 consumption. The split point balances the
            # engine rates (ACT ~1.85 ns/col single Sign pass vs DVE ~2.2
            # ns/col two-op clamp).
            HALF = 576

            def sign_piece(ks, wsrc, lo, hi, on_act, tag):
                if on_act:
                    nc.scalar.activation(sT[:, ks, :, lo:hi],
                                         wsrc[:, :, lo:hi], AF.Sign)
                else:
                    stmp = wstream.tile([P, 2, hi - lo], bf16,
                                        tag=f"stmp{tag}", bufs=2,
                                        name=f"stmp{ks}_{lo}")
                    nc.vector.tensor_scalar(
                        stmp[:], wsrc[:, :, lo:hi], 1e30, 1.0,
                        mybir.AluOpType.mult, mybir.AluOpType.min)
                    nc.vector.tensor_scalar(
                        sT[:, ks, :, lo:hi], stmp[:], -1.0, None,
                        mybir.AluOpType.max)

            for g, (wt, k0, wq) in enumerate(wts):
                for s in range(wq):
                    ks = k0 + s
                    wsrc = wt[:, 2 * s:2 * s + 2, :]
                    if ks == 0:
                        # ks0 in quarters on both engines: the first
                        # stationary piece gates the first real matmul, and
                        # ks1+ production has slack, so the extra engine
                        # time here is free
                        sign_piece(ks, wsrc, 0, 256, True, "q")
                        sign_piece(ks, wsrc, 576, 832, False, "q")
                        sign_piece(ks, wsrc, 256, 576, True, "q")
                        sign_piece(ks, wsrc, 832, 1024, False, "q")
                    else:
                        sign_piece(ks, wsrc, 0, HALF, True, "h")
                        sign_piece(ks, wsrc, HALF, OSH, False, "h")
            # |W| partials pinned past the sign stream (tile_wait_until):
            # they are ready before the DVE sign ops and would otherwise be
            # scheduled ahead of them; the scale isn't needed until span
            # EARLY's evictions (~50 us), so a 16 us floor costs nothing.
            with tc.tile_wait_until(0.016):
                for g, (wt, k0, wq) in enumerate(wts):
                    nc.vector.tensor_reduce(absacc[:, g:g + 1],
                                            wt[:, :2 * wq, :],
                                            axis=mybir.AxisListType.XY,
                                            op=mybir.AluOpType.add,
                                            apply_absolute_value=True)

            # --- global scale via AllReduce of the scalar partial ---
            # per-chunk |W| abs-row-sums live in absacc; fold the chunk axis
            # on DVE, then the partition axis via a DRAM bounce (the
            # partition axis can't fold into an SBUF free axis directly).
            # Scale is only needed by span EARLY's evictions (~50 us in).
            colsum = small.tile([P, 1], f32)
            nc.vector.reduce_sum(colsum[:], absacc[:],
                                 axis=mybir.AxisListType.X)
            nc.gpsimd.dma_start(colsum_dram.ap(), colsum[:, 0])
            rowt = small.tile([1, P], f32)
            nc.gpsimd.dma_start(rowt[0:1, :], colsum_dram.ap()[None, :])
            part = small.tile([1, 8], f32)
            nc.vector.memset(part[:], 0.0)
            nc.vector.reduce_sum(part[0:1, 0:1], rowt[0:1, :],
                                 axis=mybir.AxisListType.X)
            nc.gpsimd.dma_start(cc_in.ap(), part[:])
            if use_collective:
                nc.gpsimd.collective_compute(
                    "AllReduce",
                    mybir.AluOpType.add,
                    replica_groups=[list(range(NCORES))],
                    ins=[cc_in.ap()],
                    outs=[cc_out.ap()],
                )
                cc_result = cc_out
            else:
                # timing-model variant (TimelineSim can't model collectives):
                # local partial stands in for the global sum
                nc.gpsimd.dma_start(cc_out.ap(), cc_in.ap())
                cc_result = cc_out
            # broadcast the reduced scalar to all 128 partitions straight
            # from DRAM (step-0 source AP)
            scale_raw = small.tile([P, 1], f32)
            with nc.allow_non_contiguous_dma(reason="scale broadcast"):
                nc.gpsimd.dma_start(scale_raw[:, 0:1],
                                    cc_result.ap()[0:1, 0:1].to_broadcast((P, 1)))
            scale_b = small.tile([P, 1], f32)
            nc.scalar.activation(scale_b[:], scale_raw[:], AF.Copy,
                                 scale=1.0 / float(D_OUT * D_IN * W_PRESCALE))

            # --- main GEMM: outT[o, t] = sum_k sT[k, o] * (xhi+xlo)[k, t] ---
            # DoubleRow fp8: each matmul contracts 256 k (2 planes x 128
            # partitions) at 0.5 cycles per output column. hi and lo input
            # planes accumulate into the same PSUM bank; the sign stationary
            # is shared by both per (ks, o).
            spans = [(q + r * NSPAN, (q % NSPAN) * SPAN)
                     for r in range(repeat) for q in range(NSPAN)]
            nlo = KS - n_skip_lo

            def mm(ps, o, ks, pi, src, nparts):
                nc.tensor.matmul(
                    ps[:], sT[:, ks, :, o * P:(o + 1) * P], src[:],
                    start=(ks == 0 and pi == 0),
                    stop=(ks == KS - 1 and pi == nparts - 1),
                    perf_mode=DR,
                )

            def evict(stage, ps, o, early):
                if early:
                    # scale may still be in flight (the AllReduce) - plain
                    # copy frees the PSUM bank now; scale+bias folded in a
                    # DVE pass afterwards.
                    nc.scalar.activation(stage[:, o, :], ps[:], AF.Copy)
                    nc.vector.tensor_scalar(
                        stage[:, o, :], stage[:, o, :],
                        scale_b[:, 0:1], bias_sb[:, o:o + 1],
                        mybir.AluOpType.mult, mybir.AluOpType.add)
                else:
                    nc.scalar.activation(
                        stage[:, o, :], ps[:], AF.Identity,
                        bias=bias_sb[:, o:o + 1], scale=scale_b[:, 0:1])

            for q, t0 in spans:
                early = q < EARLY
                if q == 0:
                    inq = in0
                else:
                    inq = [issue_in(q, t0, ks) for ks in range(KS)]
                stage = outpool.tile([P, OT, SPAN], bf16, tag="stage",
                                     name=f"st{q}")
                psums = [pmm.tile([P, SPAN], f32, tag="mm",
                                  name=f"pp{q}_{o}") for o in range(OT)]
                if q < 2:
                    # ks-outer: consume each sign plane / input tile as it's
                    # produced; all 8 PSUM banks accumulate simultaneously.
                    # At the last ks, finish + evict per o so banks free for
                    # the next span as the ACT engine catches up.
                    for ks in range(KS - 1):
                        for o in range(OT):
                            for pi, src in enumerate(
                                    s for s in inq[ks] if s is not None):
                                mm(psums[o], o, ks, pi, src,
                                   2 if ks < nlo else 1)
                    ks = KS - 1
                    for o in range(OT):
                        for pi, src in enumerate(
                                s for s in inq[ks] if s is not None):
                            mm(psums[o], o, ks, pi, src, 2 if ks < nlo else 1)
                        evict(stage, psums[o], o, early)
                else:
                    last = q == spans[-1][0]
                    for o in range(OT):
                        for ks in range(KS):
                            for pi, src in enumerate(
                                    s for s in inq[ks] if s is not None):
                                mm(psums[o], o, ks, pi, src,
                                   2 if ks < nlo else 1)
                        evict(stage, psums[o], o, early)
                        if last:
                            # per-o stores right behind each eviction, on
                            # the SP ring - input loads are done by now, so
                            # SP is idle and the ACT queue keeps evicting:
                            # the drain tail is one small DMA
                            nc.sync.dma_start(outT_r[:, o, t0:t0 + SPAN],
                                              stage[:, o, :])
                    if last:
                        continue
                # batched stores per span half on the SWDGE queue (two
                # ~1.5 us device slices interleave with input loads better
                # than one 3 us one)
                h = OT // 2
                nc.gpsimd.dma_start(outT_r[:, 0:h, t0:t0 + SPAN],
                                    stage[:, 0:h, :])
                nc.gpsimd.dma_start(outT_r[:, h:, t0:t0 + SPAN],
                                    stage[:, h:, :])

    if dedup_ldw:
        _dedup_ldweights(nc, mybir)
    nc.compile()
    return nc


def _dedup_ldweights(nc, mybir):
    """Drop consecutive InstLdweights that reload the exact same stationary
    AP with only matmuls in between. Tile emits one weight load per matmul
    even when several matmuls share a stationary; the following
    non-self-loading matmuls keep using the already-loaded array state.
    Only waitless/updateless loads are removed."""
    removed = 0
    for bb in nc.m.functions[0].blocks:
        il = bb.instructions
        kept = []
        prev_sig = None
        for i in il:
            if isinstance(i, mybir.InstLdweights):
                sig = str(i.ins[0])
                if (sig == prev_sig and not i.has_wait()
                        and not i.has_update()):
                    nc.inst_map.pop(i.name, None)
                    removed += 1
                    continue
                prev_sig = sig
            elif isinstance(i, mybir.InstMatmult):
                pass
            elif getattr(i, "engine", None) == mybir.EngineType.PE:
                prev_sig = None
            kept.append(i)
        il[:] = kept


def _get_nc():
    if "nc" not in _NC_CACHE:
        _NC_CACHE["nc"] = _build_nc()
    return _NC_CACHE["nc"]


def _make_in_maps(input, weight, bias):
    inT = np.ascontiguousarray(input.T)
    inT_hi = inT.astype(ml_dtypes.float8_e4m3)
    inT_lo = (inT - inT_hi.astype(np.float32)).astype(ml_dtypes.float8_e4m3)
    wT_full = weight.T  # [D_IN, D_OUT] view
    in_maps = []
    for j in range(NCORES):
        bsh = bias[j * OSH:(j + 1) * OSH]
        in_maps.append({
            "inH": inT_hi,
            "inL": inT_lo,
            "wT": (np.ascontiguousarray(wT_full[:, j * OSH:(j + 1) * OSH])
                   * np.float32(W_PRESCALE)).astype(ml_dtypes.float8_e4m3),
            "bias2d": np.ascontiguousarray(
                bsh.reshape(OT, P).T, dtype=np.float32),
        })
    return in_maps


def run(input, weight, bias, trace=False, **spmd_kwargs):
    from concourse.bass_utils import run_bass_kernel_spmd

    nc = _get_nc()
    in_maps = _make_in_maps(np.asarray(input, dtype=np.float32),
                            np.asarray(weight, dtype=np.float32),
                            np.asarray(bias, dtype=np.float32))
    res = run_bass_kernel_spmd(nc, in_maps, core_ids=list(range(NCORES)),
                               trace=trace, **spmd_kwargs)
    outT = np.concatenate([r["outT"] for r in res.results], axis=0)
    out = np.ascontiguousarray(outT.T.astype(np.float32))
    return out, res


def kernel(input, weight, bias):
    out, _ = run(input, weight, bias, trace=False)
    return out
